# revision 11
# baseline (speedup 1.0000x reference)
"""Trainium2 Bass kernel for nn_CausalSelfAttention_60284160967096 (v4).

Sharding: 8 cores = 2 (batch) x 4 (kv-head groups).  Each core computes its
batch's attention for one kv-head (4 query heads), the Gram-Schmidt (_xsa)
correction, then an AllGather of y within the 4-core group and a row-sharded
output projection producing a 512-column slice of the output.

The axon tunnel (~44 MB/s) dominates wall time, so wire bytes are minimized:
  - x / weights / output cross the wire in bf16
  - x is shipped as per-core feature quarters in natural [T, 512] layout
    (16 MB total instead of 4x-duplicated 128 MB) and AllGathered on-device
    after an on-device PE transpose
  - weight slices are packed into one [1280, D] block per head group; each
    core ships HALF of it and an AllGather over (b=0,b=1) pairs rebuilds the
    full block (21 MB instead of 80 MB)
  - rope tables / causal mask / identity are inline_tensor NEFF constants
    (zero wire bytes)
The ternary-quantized *effective* weights are computed on the host in f32
(bitwise-identical quantization decisions to the reference; device-side
quantization of bf16-rounded weights flips ~0.14% of ternary decisions and
costs ~2% rel error).  SDPA/rmsnorm/rope/_xsa stay fp32/fp32r on device;
only wire-adjacent tensors are bf16.
"""

import numpy as np

import jax

import concourse.bass as bass
import concourse.bass_isa as bass_isa
import concourse.mybir as mybir
import concourse.tile as tile
from concourse import bacc, bass_utils

# Persistent XLA compilation cache: run_bass_kernel_spmd builds a fresh jit
# closure per call, which otherwise re-pays ~1s of XLA compile every call.
try:
    jax.config.update("jax_compilation_cache_dir", "/tmp/jax_cc_nnattn")
    jax.config.update("jax_persistent_cache_min_compile_time_secs", 0.0)
    jax.config.update("jax_persistent_cache_min_entry_size_bytes", 0)
except Exception:
    pass

F32 = mybir.dt.float32
F32R = mybir.dt.float32r
BF16 = mybir.dt.bfloat16
FP16 = mybir.dt.float16
I8 = mybir.dt.int8
NPBF16 = mybir.dt.np(BF16)
NPFP16 = mybir.dt.np(FP16)
AF = mybir.ActivationFunctionType
OP = mybir.AluOpType

T = 2048
D = 2048
HD = 128
NQ = 4          # query heads per core
TB = 512        # token block
NTB = T // TB   # 4
KT = D // 128   # 16 contraction tiles
ST = T // 128   # 16 s tiles
N_CORES = 8
WROWS = NQ * HD + HD + HD + NQ * HD   # 1280 packed weight rows
RMS_EPS = 1.1920928955078125e-07
INV_SQRT_HD = float(np.float32(1.0) / np.sqrt(np.float32(HD)))
NEG_BIG = -1.0e30


def _host_constants():
    t = np.arange(T, dtype=np.float32)
    inv_freq = (1.0 / 10000.0 ** (np.arange(0, HD, 2, dtype=np.float32) / HD))
    freqs = np.outer(t, inv_freq).astype(np.float32)        # [T, 64]
    cos_h = np.cos(freqs).T.astype(np.float32)              # [64, T]
    sin_h = np.sin(freqs).T.astype(np.float32)
    cosT = np.ascontiguousarray(np.concatenate([cos_h, cos_h], axis=0))
    sinT = np.ascontiguousarray(np.concatenate([sin_h, -sin_h], axis=0))
    s = np.arange(128)[:, None]
    u = np.arange(896)[None, :]
    maskadd = np.where(u >= s + 384, 0.0, NEG_BIG).astype(np.float32)
    ident = np.eye(128, dtype=np.float32)
    return cosT, sinT, maskadd, ident


def _build_nc():
    nc = bacc.Bacc("TRN2", target_bir_lowering=False, debug=False,
                   num_devices=N_CORES)

    # per-core external inputs: x bf16, weights int8 codes + fold scales
    xqd = nc.dram_tensor("xq", [T, 512], BF16, kind="ExternalInput")
    wpd = nc.dram_tensor("wpack", [WROWS // 2, D], I8, kind="ExternalInput")
    qgaind = nc.dram_tensor("qgain", [1, NQ], F32, kind="ExternalInput")
    scld = nc.dram_tensor("scl", [1, 2], F32, kind="ExternalInput")
    # int8 output codes + per-token f32 scales (halves fetch + zero-buffer
    # wire; the output is heavy-tailed per row, so scales go per token)
    outc = nc.dram_tensor("outc", [NQ * HD, T], I8, kind="ExternalOutput")
    oscld = nc.dram_tensor("oscl", [1, T], F32, kind="ExternalOutput")

    # NEFF-embedded rope tables (no wire bytes; fp16 keeps the BIR small,
    # rounding is ~2e-4).  Mask and identities are generated on device.
    cos_np, sin_np, _mask_np, _ident_np = _host_constants()
    cosd = nc.inline_tensor(cos_np.astype(NPFP16), name="cosT")
    sind = nc.inline_tensor(sin_np.astype(NPFP16), name="sinT")

    with nc.allow_low_precision(reason="bf16 wire + fp32r matmul pipeline"), \
         tile.TileContext(nc) as tc:
        with (
            tc.tile_pool(name="const", bufs=1) as constp,
            tc.tile_pool(name="acts", bufs=1) as actp,
            tc.tile_pool(name="weights", bufs=1) as wp,
            tc.tile_pool(name="psum_acc", bufs=6, space="PSUM") as psum_acc,
            tc.tile_pool(name="psum_small", bufs=2, space="PSUM") as psum_small,
            tc.tile_pool(name="dram", bufs=1, space="DRAM") as dramp,
        ):
            # ---- constants ----
            onesf = constp.tile([128, 1], F32)
            nc.vector.memset(onesf[:], 1.0)
            ones128 = constp.tile([128, 1], F32R)
            nc.scalar.copy(ones128[:], onesf[:])
            # causal mask: keep 0 where u >= s + 384, else NEG_BIG
            mask = constp.tile([128, 896], F32)
            nc.gpsimd.memset(mask[:], 0.0)
            nc.gpsimd.affine_select(out=mask[:], in_=mask[:],
                                    pattern=[[1, 896]], base=-384,
                                    channel_multiplier=-1,
                                    compare_op=OP.is_ge, fill=NEG_BIG)
            cosh = constp.tile([HD, T], FP16)
            nc.sync.dma_start(out=cosh[:], in_=cosd[:])
            cosb = constp.tile([HD, T], F32)
            nc.vector.tensor_copy(cosb[:], cosh[:])
            sinh = constp.tile([HD, T], FP16)
            nc.sync.dma_start(out=sinh[:], in_=sind[:])
            sinb = constp.tile([HD, T], F32)
            nc.vector.tensor_copy(sinb[:], sinh[:])
            # identities (transpose operands): diag(1) via affine_select
            onesb2 = constp.tile([128, 128], BF16)
            nc.vector.memset(onesb2[:], 1.0)
            identb = constp.tile([128, 128], BF16)
            nc.gpsimd.affine_select(out=identb[:], in_=onesb2[:],
                                    pattern=[[1, 128]], base=0,
                                    channel_multiplier=-1,
                                    compare_op=OP.is_equal, fill=0.0)
            onesf2 = constp.tile([128, 128], F32)
            nc.vector.memset(onesf2[:], 1.0)
            identf = constp.tile([128, 128], F32)
            nc.gpsimd.affine_select(out=identf[:], in_=onesf2[:],
                                    pattern=[[1, 128]], base=0,
                                    channel_multiplier=-1,
                                    compare_op=OP.is_equal, fill=0.0)
            qgain = constp.tile([1, NQ], F32)
            nc.sync.dma_start(out=qgain[:], in_=qgaind[:])
            scl = constp.tile([1, 2], F32)
            nc.sync.dma_start(out=scl[:], in_=scld[:])
            svb = constp.tile([128, 1], F32)
            nc.gpsimd.partition_broadcast(svb[:], scl[0:1, 0:1])
            spb = constp.tile([128, 1], F32)
            nc.gpsimd.partition_broadcast(spb[:], scl[0:1, 1:2])
            eps1 = constp.tile([1, 1], F32)
            nc.vector.memset(eps1[:], RMS_EPS)

            # ---- weight AllGather across the (b=0, b=1) pair ----
            wtb = dramp.tile([WROWS // 2, D], I8, name="wtb")
            wfull = dramp.tile([WROWS, D], I8, name="wfull")
            nc.sync.dma_start(out=wtb[:], in_=wpd[:])
            nc.gpsimd.collective_compute(
                "AllGather", OP.bypass,
                replica_groups=[[0, 4], [1, 5], [2, 6], [3, 7]],
                ins=[wtb[:].opt()], outs=[wfull[:].opt()])

            # ---- x transpose (on-device) + AllGather across head groups ----
            xtb = dramp.tile([512, T], BF16, name="xtb")
            xTd = dramp.tile([D, T], BF16, name="xTd")
            with tc.tile_pool(name="xtr", bufs=1) as xtrp:
                xTq = [xtrp.tile([128, T], BF16, name=f"xTq{fc}",
                                 tag=f"xTq{fc}") for fc in range(4)]
                for tr in range(ST):
                    xt_in = xtrp.tile([128, 512], BF16, name="xt_in",
                                      tag="xt_in", bufs=4)
                    nc.sync.dma_start(out=xt_in[:],
                                      in_=xqd[128 * tr:128 * (tr + 1), :])
                    for fc in range(4):
                        ps_t = psum_acc.tile([128, 128], BF16, name="ps_xt",
                                             tag="acc")
                        nc.tensor.transpose(
                            ps_t[:], xt_in[:, 128 * fc:128 * (fc + 1)],
                            identb[:])
                        nc.vector.tensor_copy(
                            xTq[fc][:, 128 * tr:128 * (tr + 1)], ps_t[:])
                for fc in range(4):
                    nc.sync.dma_start(out=xtb[128 * fc:128 * (fc + 1), :],
                                      in_=xTq[fc][:])
            nc.gpsimd.collective_compute(
                "AllGather", OP.bypass,
                replica_groups=[[0, 1, 2, 3], [4, 5, 6, 7]],
                ins=[xtb[:].opt()], outs=[xTd[:].opt()])

            # ---- weight transpose straight into effective weight tiles ----
            # wfull rows: q 0:512, k 512:640, v 640:768, p 768:1280
            wq_t = [wp.tile([128, NQ * HD], BF16, name=f"wq{ck}",
                            tag=f"wq{ck}") for ck in range(KT)]
            wk_t = [wp.tile([128, HD], BF16, name=f"wk{ck}", tag=f"wk{ck}")
                    for ck in range(KT)]
            wv_t = [wp.tile([128, HD], BF16, name=f"wv{ck}", tag=f"wv{ck}")
                    for ck in range(KT)]
            wp_t = [wp.tile([128, NQ * HD], BF16, name=f"wpj{ck}",
                            tag=f"wpj{ck}") for ck in range(KT)]
            with tc.tile_pool(name="wtr", bufs=1) as wtrp:
                for rt in range(10):
                    w_i8 = wtrp.tile([128, D], I8, name="w_i8",
                                     tag="w_i8", bufs=3)
                    nc.sync.dma_start(
                        out=w_i8[:], in_=wfull[128 * rt:128 * (rt + 1), :])
                    w_in = wtrp.tile([128, D], BF16, name="w_in",
                                     tag="w_in", bufs=3)
                    nc.vector.tensor_copy(w_in[:], w_i8[:])
                    if rt < 4:
                        dst, r = wq_t, rt
                    elif rt == 4:
                        dst, r = wk_t, 0
                    elif rt == 5:
                        dst, r = wv_t, 0
                    else:
                        dst, r = wp_t, rt - 6
                    for ck in range(KT):
                        ps_t = psum_acc.tile([128, 128], BF16, name="ps_wt",
                                             tag="acc")
                        nc.tensor.transpose(
                            ps_t[:], w_in[:, 128 * ck:128 * (ck + 1)],
                            identb[:])
                        nc.vector.tensor_copy(
                            dst[ck][:, 128 * r:128 * (r + 1)], ps_t[:])

            # ---- persistent activations ----
            qf = [actp.tile([128, T], F32R, name=f"qf{h}", tag=f"qf{h}")
                  for h in range(NQ)]
            kf = actp.tile([128, T], F32R, name="kf", tag="kf")
            vT = actp.tile([128, T], F32, name="vT", tag="vT")
            vs = [actp.tile([128, 128], F32R, name=f"vs{i}", tag=f"vs{i}")
                  for i in range(ST)]

            # ---- QKV projections + rmsnorm + rope ----
            with tc.tile_pool(name="qkv_tmp", bufs=2) as tp:
                for j in range(NTB):
                    js = slice(TB * j, TB * (j + 1))
                    # stream x k-tiles for this t-block from xTd (bf16)
                    xts = []
                    for ck in range(KT):
                        xt = tp.tile([128, TB], BF16, name="xt",
                                     tag="xt", bufs=4)
                        nc.sync.dma_start(
                            out=xt[:], in_=xTd[128 * ck:128 * (ck + 1), js])
                        xts.append(xt)
                    ps_o = [psum_acc.tile([128, TB], F32, name=f"ps_o{o}",
                                          tag="acc") for o in range(6)]
                    for ck in range(KT):
                        st, sp_ = (ck == 0), (ck == KT - 1)
                        for h in range(NQ):
                            nc.tensor.matmul(
                                ps_o[h][:],
                                wq_t[ck][:, 128 * h:128 * (h + 1)],
                                xts[ck][:], start=st, stop=sp_)
                        nc.tensor.matmul(ps_o[4][:], wk_t[ck][:], xts[ck][:],
                                         start=st, stop=sp_)
                        nc.tensor.matmul(ps_o[5][:], wv_t[ck][:], xts[ck][:],
                                         start=st, stop=sp_)

                    # v: evict straight to vT, folding the int8 scale s_v
                    nc.vector.tensor_scalar(out=vT[:, js], in0=ps_o[5][:],
                                            scalar1=svb[0:128, 0:1],
                                            scalar2=None, op0=OP.mult)

                    # q heads and k: rmsnorm + rope
                    for o in range(5):
                        is_q = o < NQ
                        raw = tp.tile([128, TB], F32, name="raw", tag="raw",
                                      bufs=3)
                        nc.scalar.copy(raw[:], ps_o[o][:])
                        sq = tp.tile([128, TB], F32R, name="sq", tag="sq",
                                     bufs=2)
                        nc.vector.tensor_tensor(out=sq[:], in0=raw[:],
                                                in1=raw[:], op=OP.mult)
                        ps_r = psum_small.tile([1, TB], F32, name="ps_r",
                                               tag="small")
                        nc.tensor.matmul(ps_r[:], ones128[:], sq[:],
                                         start=True, stop=True)
                        rsq = tp.tile([1, TB], F32, name="rsq", tag="rsq",
                                      bufs=2)
                        nc.scalar.activation(rsq[:], ps_r[:], AF.Sqrt,
                                             bias=eps1[0:1, 0:1],
                                             scale=1.0 / HD)
                        rinv = tp.tile([1, TB], F32, name="rinv", tag="rinv",
                                       bufs=2)
                        nc.vector.reciprocal(rinv[:], rsq[:])
                        rsc = tp.tile([1, TB], F32R, name="rsc", tag="rsc",
                                      bufs=2)
                        if is_q:
                            nc.vector.tensor_scalar(
                                out=rsc[:], in0=rinv[:],
                                scalar1=qgain[0:1, o:o + 1], scalar2=None,
                                op0=OP.mult)
                        else:
                            nc.scalar.copy(rsc[:], rinv[:])
                        rb_s = tp.tile([128, TB], F32, name="rb_s",
                                       tag="rb_s", bufs=2)
                        nc.gpsimd.partition_broadcast(rb_s[:],
                                                      rsc[:].bitcast(F32))
                        # rope: rawsw = halves of raw swapped; sinb has -sin
                        # in its high half, so ro = raw*cos + rawsw*sin.
                        rawsw = tp.tile([128, TB], F32, name="rawsw",
                                        tag="rawsw", bufs=2)
                        nc.scalar.copy(rawsw[0:64, :], raw[64:128, :])
                        nc.scalar.copy(rawsw[64:128, :], raw[0:64, :])
                        rock = tp.tile([128, TB], F32, name="rock",
                                       tag="rock", bufs=2)
                        nc.vector.tensor_tensor(out=rock[:], in0=raw[:],
                                                in1=cosb[:, js], op=OP.mult)
                        rask = tp.tile([128, TB], F32, name="rask",
                                       tag="rask", bufs=2)
                        nc.vector.tensor_tensor(out=rask[:], in0=rawsw[:],
                                                in1=sinb[:, js], op=OP.mult)
                        ro = tp.tile([128, TB], F32, name="ro", tag="ro",
                                     bufs=2)
                        nc.vector.tensor_tensor(out=ro[:], in0=rock[:],
                                                in1=rask[:], op=OP.add)
                        dst = qf[o][:, js] if is_q else kf[:, js]
                        nc.vector.tensor_tensor(out=dst, in0=ro[:],
                                                in1=rb_s[:], op=OP.mult)

            # v transposed tiles [s, dh] for the attn@v matmul
            with tc.tile_pool(name="vtr", bufs=2) as vtrp:
                for i in range(ST):
                    ps_t = psum_acc.tile([128, 128], F32, name="ps_vt",
                                         tag="acc")
                    nc.tensor.transpose(ps_t[:], vT[:, 128 * i:128 * (i + 1)],
                                        identf[:])
                    nc.scalar.copy(vs[i][:], ps_t[:])

            # ---- SDPA + _xsa per t-block, then one AllGather + proj ----
            ybounce = dramp.tile([NQ * HD, T], BF16, name="ybounce")
            yfull = dramp.tile([4 * NQ * HD, T], BF16, name="yfull")

            with tc.tile_pool(name="sdpa", bufs=2) as sp:
                for j in range(NTB):
                    js = slice(TB * j, TB * (j + 1))
                    n_i = 4 * j + 4
                    denr = sp.tile([1, TB], F32, name="denr", tag="denr",
                                   bufs=2)
                    for h in range(NQ):
                        ps_y = psum_acc.tile([128, TB], F32, name="ps_y",
                                             tag="acc")
                        ps_z = psum_small.tile([1, TB], F32, name="ps_z",
                                               tag="small")
                        for i in range(n_i):
                            ps_s = psum_acc.tile([128, TB], F32, name="ps_s",
                                                 tag="acc")
                            nc.tensor.matmul(
                                ps_s[:], kf[:, 128 * i:128 * (i + 1)],
                                qf[h][:, js], start=True, stop=True)
                            if i >= 4 * j:
                                off = 128 * (i - 4 * j)
                                u0 = 384 - off
                                nc.vector.tensor_tensor(
                                    out=ps_s[:], in0=ps_s[:],
                                    in1=mask[:, u0:u0 + TB], op=OP.add)
                            et = sp.tile([128, TB], F32R, name="et",
                                         tag=f"et{i & 1}", bufs=2)
                            nc.scalar.activation(et[:], ps_s[:], AF.Exp,
                                                 scale=INV_SQRT_HD)
                            st, spp = (i == 0), (i == n_i - 1)
                            nc.tensor.matmul(ps_z[:], ones128[:], et[:],
                                             start=st, stop=spp,
                                             skip_group_check=True)
                            nc.tensor.matmul(ps_y[:], vs[i][:], et[:],
                                             start=st, stop=spp,
                                             skip_group_check=True)
                        # epilogue for (h, j)
                        y_h = sp.tile([128, TB], F32, name="y_h", tag="y_h",
                                      bufs=2)
                        nc.scalar.copy(y_h[:], ps_y[:])
                        if h == 0:
                            vsq = sp.tile([128, TB], F32R, name="vsq",
                                          tag="vsq", bufs=1)
                            nc.vector.tensor_tensor(out=vsq[:], in0=vT[:, js],
                                                    in1=vT[:, js],
                                                    op=OP.mult)
                            ps_d = psum_small.tile([1, TB], F32, name="ps_d",
                                                   tag="small")
                            nc.tensor.matmul(ps_d[:], ones128[:], vsq[:],
                                             start=True, stop=True)
                            den = sp.tile([1, TB], F32, name="den", tag="den",
                                          bufs=2)
                            nc.vector.tensor_scalar(out=den[:], in0=ps_d[:],
                                                    scalar1=1e-24,
                                                    scalar2=None, op0=OP.max)
                            nc.vector.reciprocal(denr[:], den[:])
                        zinv = sp.tile([1, TB], F32, name="zinv", tag="zinv",
                                       bufs=2)
                        nc.vector.reciprocal(zinv[:], ps_z[:])
                        zr = sp.tile([1, TB], F32R, name="zr", tag="zr",
                                     bufs=2)
                        nc.scalar.copy(zr[:], zinv[:])
                        yv = sp.tile([128, TB], F32R, name="yv", tag="yv",
                                     bufs=1)
                        nc.vector.tensor_tensor(out=yv[:], in0=y_h[:],
                                                in1=vT[:, js], op=OP.mult)
                        ps_dot = psum_small.tile([1, TB], F32, name="ps_dot",
                                                 tag="small")
                        nc.tensor.matmul(ps_dot[:], ones128[:], yv[:],
                                         start=True, stop=True)
                        c1 = sp.tile([1, TB], F32, name="c1", tag="c1",
                                     bufs=2)
                        nc.vector.tensor_tensor(out=c1[:], in0=ps_dot[:],
                                                in1=denr[:], op=OP.mult)
                        c2 = sp.tile([1, TB], F32R, name="c2", tag="c2",
                                     bufs=2)
                        nc.vector.tensor_tensor(out=c2[:], in0=c1[:],
                                                in1=zinv[:], op=OP.mult)
                        zb_s = sp.tile([128, TB], F32, name="zb_s",
                                       tag="zb_s", bufs=1)
                        cb_s = sp.tile([128, TB], F32, name="cb_s",
                                       tag="cb_s", bufs=1)
                        nc.gpsimd.partition_broadcast(zb_s[:],
                                                      zr[:].bitcast(F32))
                        nc.gpsimd.partition_broadcast(cb_s[:],
                                                      c2[:].bitcast(F32))
                        t1 = sp.tile([128, TB], F32, name="t1", tag="t1",
                                     bufs=1)
                        t2 = sp.tile([128, TB], F32, name="t2", tag="t2",
                                     bufs=1)
                        nc.vector.tensor_tensor(out=t1[:], in0=y_h[:],
                                                in1=zb_s[:], op=OP.mult)
                        nc.vector.tensor_tensor(out=t2[:], in0=vT[:, js],
                                                in1=cb_s[:], op=OP.mult)
                        yfin = sp.tile([128, TB], BF16, name="yfin",
                                       tag="yfin", bufs=2)
                        nc.vector.tensor_tensor(out=yfin[:], in0=t1[:],
                                                in1=t2[:], op=OP.subtract)
                        nc.sync.dma_start(
                            out=ybounce[128 * h:128 * (h + 1), js],
                            in_=yfin[:])
            nc.gpsimd.collective_compute(
                "AllGather", OP.bypass,
                replica_groups=[[0, 1, 2, 3], [4, 5, 6, 7]],
                ins=[ybounce[:].opt()], outs=[yfull[:].opt()])

            # ---- output projection (row-sharded: 512 out cols/core) ----
            # Accumulate the full f32 result in SBUF, then int8-quantize with
            # per-row scales (round-to-nearest + saturation on the convert).
            with tc.tile_pool(name="proj", bufs=2) as pp:
                ofull = [pp.tile([128, T], F32, name=f"ofull{o}",
                                 tag=f"ofull{o}", bufs=1) for o in range(4)]
                for j in range(NTB):
                    js = slice(TB * j, TB * (j + 1))
                    ps_p = [psum_acc.tile([128, TB], F32, name=f"ps_p{o}",
                                          tag="acc") for o in range(4)]
                    for ck in range(KT):
                        yt = pp.tile([128, TB], BF16, name="yt", tag="yt",
                                     bufs=4)
                        nc.sync.dma_start(
                            out=yt[:],
                            in_=yfull[128 * ck:128 * (ck + 1), js])
                        st, spp = (ck == 0), (ck == KT - 1)
                        for o in range(4):
                            nc.tensor.matmul(
                                ps_p[o][:],
                                wp_t[ck][:, 128 * o:128 * (o + 1)],
                                yt[:], start=st, stop=spp)
                    for o in range(4):
                        nc.vector.tensor_scalar(out=ofull[o][:, js],
                                                in0=ps_p[o][:],
                                                scalar1=spb[0:128, 0:1],
                                                scalar2=None, op0=OP.mult)
                # per-token absmax over all 512 out rows (partition all-
                # reduce per o-tile, then max across the 4 tiles)
                am = pp.tile([128, T], F32, name="am", tag="am", bufs=1)
                am2 = pp.tile([128, T], F32, name="am2", tag="am2", bufs=1)
                nc.gpsimd.partition_all_reduce(
                    am[:], ofull[0][:], channels=128,
                    reduce_op=bass_isa.ReduceOp.absmax)
                for o in range(1, 4):
                    nc.gpsimd.partition_all_reduce(
                        am2[:], ofull[o][:], channels=128,
                        reduce_op=bass_isa.ReduceOp.absmax)
                    nc.vector.tensor_tensor(out=am[:], in0=am[:],
                                            in1=am2[:], op=OP.max)
                nc.vector.tensor_scalar(out=am[:], in0=am[:], scalar1=1e-30,
                                        scalar2=None, op0=OP.max)
                osc = pp.tile([1, T], F32, name="osc", tag="osc", bufs=1)
                nc.vector.tensor_scalar(out=osc[:], in0=am[0:1, :],
                                        scalar1=1.0 / 127.0, scalar2=None,
                                        op0=OP.mult)
                nc.sync.dma_start(out=oscld[:], in_=osc[:])
                rsc = pp.tile([128, T], F32, name="rsc2", tag="rsc2", bufs=1)
                nc.vector.reciprocal(rsc[:], am[:])
                nc.vector.tensor_scalar(out=rsc[:], in0=rsc[:], scalar1=127.0,
                                        scalar2=None, op0=OP.mult)
                for o in range(4):
                    codes = pp.tile([128, T], I8, name="codes", tag="codes",
                                    bufs=2)
                    nc.vector.tensor_tensor(out=codes[:], in0=ofull[o][:],
                                            in1=rsc[:], op=OP.mult)
                    nc.sync.dma_start(out=outc[128 * o:128 * (o + 1), :],
                                      in_=codes[:])

    nc.compile()
    return nc


_NC = None


def _get_nc():
    global _NC
    if _NC is None:
        _NC = _build_nc()
    return _NC


_POOL = None


def _pool():
    global _POOL
    if _POOL is None:
        from concurrent.futures import ThreadPoolExecutor
        _POOL = ThreadPoolExecutor(max_workers=4)
    return _POOL


def _weight_codes(w, sf):
    """Host-side AnnealedBitLinear effective weight (f32, bitwise identical
    quantization decisions to the reference) + symmetric int8 encoding.
    Returns (codes int8, scale f32).  q/k scales never leave the host
    (rmsnorm makes q and k scale-invariant); s_v and s_p are folded back on
    device."""
    w = np.asarray(w, dtype=np.float32)
    wabs = np.abs(w)
    scale = np.clip(wabs.mean(axis=1, keepdims=True, dtype=np.float32),
                    1e-8, None).astype(np.float32)
    w_quant = np.where(wabs > 0.7 * scale,
                       np.copysign(scale, w).astype(np.float32),
                       np.float32(0.0))
    w_e = (1.0 - sf) * w + sf * w_quant
    s = np.float32(max(np.abs(w_e).max() / 127.0, 1e-30))
    codes = np.clip(np.rint(w_e * np.float32(1.0 / s)), -127, 127) \
        .astype(np.int8)
    return codes, s


def _make_in_maps(x, step_fraction, w_q, w_k, w_v, w_proj, q_gain):
    x = np.asarray(x, dtype=np.float32)
    sf = np.float32(np.asarray(step_fraction, dtype=np.float32).reshape(-1)[0])
    q_gain = np.asarray(q_gain, dtype=np.float32)
    futs = [_pool().submit(_weight_codes, w, sf)
            for w in (w_q, w_k, w_v, w_proj)]
    # overlap the x slice+cast work with the weight quantization
    def _xq(b, h):
        return np.ascontiguousarray(
            x[b][:, 512 * h:512 * (h + 1)]).astype(NPBF16)
    xq_futs = [_pool().submit(_xq, c // 4, c % 4) for c in range(N_CORES)]
    (wq_c, _), (wk_c, _), (wv_c, s_v), (wp_c, s_p) = \
        [f.result() for f in futs]
    scl = np.array([[s_v, s_p]], dtype=np.float32)
    # packed per-head-group weight code blocks [1280, D] int8
    wpacks = []
    for h in range(4):
        wpacks.append(np.concatenate([
            wq_c[512 * h:512 * (h + 1), :],
            wk_c[128 * h:128 * (h + 1), :],
            wv_c[128 * h:128 * (h + 1), :],
            wp_c[512 * h:512 * (h + 1), :]], axis=0))
    in_maps = []
    half = WROWS // 2
    for c in range(N_CORES):
        b, h = divmod(c, 4)
        in_maps.append({
            "xq": xq_futs[c].result(),
            "wpack": np.ascontiguousarray(
                wpacks[h][half * b:half * (b + 1), :]),
            "qgain": np.ascontiguousarray(q_gain[4 * h:4 * (h + 1)]
                                          .reshape(1, NQ)),
            "scl": scl,
        })
    return in_maps


def _assemble(results):
    out = np.empty((2, T, D), dtype=np.float32)

    def _decode(c):
        b, h = divmod(c, 4)
        codes = results[c]["outc"]            # [512, T] int8
        scl = results[c]["oscl"]              # [1, T] f32 per-token scale
        dec = np.multiply(codes, scl, dtype=np.float32)
        out[b][:, 512 * h:512 * (h + 1)] = dec.T

    list(_pool().map(_decode, range(N_CORES)))
    return out


def kernel(**inputs) -> np.ndarray:
    nc = _get_nc()
    in_maps = _make_in_maps(**inputs)
    res = bass_utils.run_bass_kernel_spmd(nc, in_maps,
                                          core_ids=list(range(N_CORES)))
    return _assemble(res.results)


def bench(**inputs):
    """Returns (output, BassKernelResults); tracing if the env supports it."""
    nc = _get_nc()
    in_maps = _make_in_maps(**inputs)
    try:
        res = bass_utils.run_bass_kernel_spmd(nc, in_maps,
                                              core_ids=list(range(N_CORES)),
                                              trace=True)
    except ModuleNotFoundError:
        res = bass_utils.run_bass_kernel_spmd(nc, in_maps,
                                              core_ids=list(range(N_CORES)))
    return _assemble(res.results), res


# revision 14
# speedup vs baseline: 1.0108x; 1.0108x over previous
"""Trainium2 Bass kernel for nn_CausalSelfAttention_60284160967096 (v4).

Sharding: 8 cores = 2 (batch) x 4 (kv-head groups).  Each core computes its
batch's attention for one kv-head (4 query heads), the Gram-Schmidt (_xsa)
correction, then an AllGather of y within the 4-core group and a row-sharded
output projection producing a 512-column slice of the output.

The axon tunnel (~44 MB/s) dominates wall time, so wire bytes are minimized:
  - x / weights / output cross the wire in bf16
  - x is shipped as per-core feature quarters in natural [T, 512] layout
    (16 MB total instead of 4x-duplicated 128 MB) and AllGathered on-device
    after an on-device PE transpose
  - weight slices are packed into one [1280, D] block per head group; each
    core ships HALF of it and an AllGather over (b=0,b=1) pairs rebuilds the
    full block (21 MB instead of 80 MB)
  - rope tables / causal mask / identity are inline_tensor NEFF constants
    (zero wire bytes)
The ternary-quantized *effective* weights are computed on the host in f32
(bitwise-identical quantization decisions to the reference; device-side
quantization of bf16-rounded weights flips ~0.14% of ternary decisions and
costs ~2% rel error).  SDPA/rmsnorm/rope/_xsa stay fp32/fp32r on device;
only wire-adjacent tensors are bf16.
"""

import numpy as np

import jax

import concourse.bass as bass
import concourse.bass_isa as bass_isa
import concourse.mybir as mybir
import concourse.tile as tile
from concourse import bacc, bass_utils

# Persistent XLA compilation cache: run_bass_kernel_spmd builds a fresh jit
# closure per call, which otherwise re-pays ~1s of XLA compile every call.
try:
    jax.config.update("jax_compilation_cache_dir", "/tmp/jax_cc_nnattn")
    jax.config.update("jax_persistent_cache_min_compile_time_secs", 0.0)
    jax.config.update("jax_persistent_cache_min_entry_size_bytes", 0)
except Exception:
    pass

F32 = mybir.dt.float32
F32R = mybir.dt.float32r
BF16 = mybir.dt.bfloat16
FP16 = mybir.dt.float16
I8 = mybir.dt.int8
NPBF16 = mybir.dt.np(BF16)
NPFP16 = mybir.dt.np(FP16)
AF = mybir.ActivationFunctionType
OP = mybir.AluOpType

T = 2048
D = 2048
HD = 128
NQ = 4          # query heads per core
TB = 512        # token block
NTB = T // TB   # 4
KT = D // 128   # 16 contraction tiles
ST = T // 128   # 16 s tiles
N_CORES = 8
WROWS = NQ * HD + HD + HD + NQ * HD   # 1280 packed weight rows
RMS_EPS = 1.1920928955078125e-07
INV_SQRT_HD = float(np.float32(1.0) / np.sqrt(np.float32(HD)))
NEG_BIG = -1.0e30


def _host_constants():
    t = np.arange(T, dtype=np.float32)
    inv_freq = (1.0 / 10000.0 ** (np.arange(0, HD, 2, dtype=np.float32) / HD))
    freqs = np.outer(t, inv_freq).astype(np.float32)        # [T, 64]
    cos_h = np.cos(freqs).T.astype(np.float32)              # [64, T]
    sin_h = np.sin(freqs).T.astype(np.float32)
    cosT = np.ascontiguousarray(np.concatenate([cos_h, cos_h], axis=0))
    sinT = np.ascontiguousarray(np.concatenate([sin_h, -sin_h], axis=0))
    s = np.arange(128)[:, None]
    u = np.arange(896)[None, :]
    maskadd = np.where(u >= s + 384, 0.0, NEG_BIG).astype(np.float32)
    ident = np.eye(128, dtype=np.float32)
    return cosT, sinT, maskadd, ident


def _build_nc():
    nc = bacc.Bacc("TRN2", target_bir_lowering=False, debug=False,
                   num_devices=N_CORES)

    # per-core external inputs: x bf16, weights int8 codes + fold scales
    xqd = nc.dram_tensor("xq", [T, 512], BF16, kind="ExternalInput")
    wpd = nc.dram_tensor("wpack", [WROWS // 2, D], I8, kind="ExternalInput")
    qgaind = nc.dram_tensor("qgain", [1, NQ], F32, kind="ExternalInput")
    scld = nc.dram_tensor("scl", [1, 2], F32, kind="ExternalInput")
    # int8 output codes + per-token f32 scales (halves fetch + zero-buffer
    # wire; the output is heavy-tailed per row, so scales go per token)
    outc = nc.dram_tensor("outc", [NQ * HD, T], I8, kind="ExternalOutput")
    oscld = nc.dram_tensor("oscl", [1, T], F32, kind="ExternalOutput")

    # NEFF-embedded rope tables (no wire bytes; fp16 keeps the BIR small,
    # rounding is ~2e-4).  Mask and identities are generated on device.
    cos_np, sin_np, _mask_np, _ident_np = _host_constants()
    cosd = nc.inline_tensor(cos_np.astype(NPFP16), name="cosT")
    sind = nc.inline_tensor(sin_np.astype(NPFP16), name="sinT")

    with nc.allow_low_precision(reason="bf16 wire + fp32r matmul pipeline"), \
         tile.TileContext(nc) as tc:
        with (
            tc.tile_pool(name="const", bufs=1) as constp,
            tc.tile_pool(name="acts", bufs=1) as actp,
            tc.tile_pool(name="weights", bufs=1) as wp,
            tc.tile_pool(name="psum_acc", bufs=6, space="PSUM") as psum_acc,
            tc.tile_pool(name="psum_small", bufs=2, space="PSUM") as psum_small,
            tc.tile_pool(name="dram", bufs=1, space="DRAM") as dramp,
        ):
            # ---- constants ----
            onesf = constp.tile([128, 1], F32)
            nc.vector.memset(onesf[:], 1.0)
            ones128 = constp.tile([128, 1], F32R)
            nc.scalar.copy(ones128[:], onesf[:])
            # causal mask: keep 0 where u >= s + 384, else NEG_BIG
            mask = constp.tile([128, 896], F32)
            nc.gpsimd.memset(mask[:], 0.0)
            nc.gpsimd.affine_select(out=mask[:], in_=mask[:],
                                    pattern=[[1, 896]], base=-384,
                                    channel_multiplier=-1,
                                    compare_op=OP.is_ge, fill=NEG_BIG)
            cosh = constp.tile([HD, T], FP16)
            nc.sync.dma_start(out=cosh[:], in_=cosd[:])
            cosb = constp.tile([HD, T], F32)
            nc.vector.tensor_copy(cosb[:], cosh[:])
            sinh = constp.tile([HD, T], FP16)
            nc.sync.dma_start(out=sinh[:], in_=sind[:])
            sinb = constp.tile([HD, T], F32)
            nc.vector.tensor_copy(sinb[:], sinh[:])
            # identities (transpose operands): diag(1) via affine_select
            onesb2 = constp.tile([128, 128], BF16)
            nc.vector.memset(onesb2[:], 1.0)
            identb = constp.tile([128, 128], BF16)
            nc.gpsimd.affine_select(out=identb[:], in_=onesb2[:],
                                    pattern=[[1, 128]], base=0,
                                    channel_multiplier=-1,
                                    compare_op=OP.is_equal, fill=0.0)
            onesf2 = constp.tile([128, 128], F32)
            nc.vector.memset(onesf2[:], 1.0)
            identf = constp.tile([128, 128], F32)
            nc.gpsimd.affine_select(out=identf[:], in_=onesf2[:],
                                    pattern=[[1, 128]], base=0,
                                    channel_multiplier=-1,
                                    compare_op=OP.is_equal, fill=0.0)
            qgain = constp.tile([1, NQ], F32)
            nc.sync.dma_start(out=qgain[:], in_=qgaind[:])
            scl = constp.tile([1, 2], F32)
            nc.sync.dma_start(out=scl[:], in_=scld[:])
            svb = constp.tile([128, 1], F32)
            nc.gpsimd.partition_broadcast(svb[:], scl[0:1, 0:1])
            spb = constp.tile([128, 1], F32)
            nc.gpsimd.partition_broadcast(spb[:], scl[0:1, 1:2])
            eps1 = constp.tile([1, 1], F32)
            nc.vector.memset(eps1[:], RMS_EPS)

            # ---- weight AllGather across the (b=0, b=1) pair ----
            wtb = dramp.tile([WROWS // 2, D], I8, name="wtb")
            wfull = dramp.tile([WROWS, D], I8, name="wfull")
            nc.sync.dma_start(out=wtb[:], in_=wpd[:])
            nc.gpsimd.collective_compute(
                "AllGather", OP.bypass,
                replica_groups=[[0, 4], [1, 5], [2, 6], [3, 7]],
                ins=[wtb[:].opt()], outs=[wfull[:].opt()])

            # ---- x transpose (on-device) + AllGather across head groups ----
            xtb = dramp.tile([512, T], BF16, name="xtb")
            xTd = dramp.tile([D, T], BF16, name="xTd")
            with tc.tile_pool(name="xtr", bufs=1) as xtrp:
                xTq = [xtrp.tile([128, T], BF16, name=f"xTq{fc}",
                                 tag=f"xTq{fc}") for fc in range(4)]
                for tr in range(ST):
                    xt_in = xtrp.tile([128, 512], BF16, name="xt_in",
                                      tag="xt_in", bufs=4)
                    nc.sync.dma_start(out=xt_in[:],
                                      in_=xqd[128 * tr:128 * (tr + 1), :])
                    for fc in range(4):
                        ps_t = psum_acc.tile([128, 128], BF16, name="ps_xt",
                                             tag="acc")
                        nc.tensor.transpose(
                            ps_t[:], xt_in[:, 128 * fc:128 * (fc + 1)],
                            identb[:])
                        nc.vector.tensor_copy(
                            xTq[fc][:, 128 * tr:128 * (tr + 1)], ps_t[:])
                for fc in range(4):
                    nc.sync.dma_start(out=xtb[128 * fc:128 * (fc + 1), :],
                                      in_=xTq[fc][:])
            nc.gpsimd.collective_compute(
                "AllGather", OP.bypass,
                replica_groups=[[0, 1, 2, 3], [4, 5, 6, 7]],
                ins=[xtb[:].opt()], outs=[xTd[:].opt()])

            # ---- weight transpose straight into effective weight tiles ----
            # wfull rows: q 0:512, k 512:640, v 640:768, p 768:1280
            wq_t = [wp.tile([128, NQ * HD], BF16, name=f"wq{ck}",
                            tag=f"wq{ck}") for ck in range(KT)]
            wk_t = [wp.tile([128, HD], BF16, name=f"wk{ck}", tag=f"wk{ck}")
                    for ck in range(KT)]
            wv_t = [wp.tile([128, HD], BF16, name=f"wv{ck}", tag=f"wv{ck}")
                    for ck in range(KT)]
            wp_t = [wp.tile([128, NQ * HD], BF16, name=f"wpj{ck}",
                            tag=f"wpj{ck}") for ck in range(KT)]
            with tc.tile_pool(name="wtr", bufs=1) as wtrp:
                for rt in range(10):
                    w_i8 = wtrp.tile([128, D], I8, name="w_i8",
                                     tag="w_i8", bufs=3)
                    nc.sync.dma_start(
                        out=w_i8[:], in_=wfull[128 * rt:128 * (rt + 1), :])
                    w_in = wtrp.tile([128, D], BF16, name="w_in",
                                     tag="w_in", bufs=3)
                    nc.vector.tensor_copy(w_in[:], w_i8[:])
                    if rt < 4:
                        dst, r = wq_t, rt
                    elif rt == 4:
                        dst, r = wk_t, 0
                    elif rt == 5:
                        dst, r = wv_t, 0
                    else:
                        dst, r = wp_t, rt - 6
                    for ck in range(KT):
                        ps_t = psum_acc.tile([128, 128], BF16, name="ps_wt",
                                             tag="acc")
                        nc.tensor.transpose(
                            ps_t[:], w_in[:, 128 * ck:128 * (ck + 1)],
                            identb[:])
                        nc.vector.tensor_copy(
                            dst[ck][:, 128 * r:128 * (r + 1)], ps_t[:])

            # ---- persistent activations ----
            qf = [actp.tile([128, T], F32R, name=f"qf{h}", tag=f"qf{h}")
                  for h in range(NQ)]
            kf = actp.tile([128, T], F32R, name="kf", tag="kf")
            vT = actp.tile([128, T], F32, name="vT", tag="vT")
            vs = [actp.tile([128, 128], F32R, name=f"vs{i}", tag=f"vs{i}")
                  for i in range(ST)]

            # ---- QKV projections + rmsnorm + rope ----
            with tc.tile_pool(name="qkv_tmp", bufs=2) as tp:
                for j in range(NTB):
                    js = slice(TB * j, TB * (j + 1))
                    # stream x k-tiles for this t-block from xTd (bf16)
                    xts = []
                    for ck in range(KT):
                        xt = tp.tile([128, TB], BF16, name="xt",
                                     tag="xt", bufs=4)
                        nc.sync.dma_start(
                            out=xt[:], in_=xTd[128 * ck:128 * (ck + 1), js])
                        xts.append(xt)
                    ps_o = [psum_acc.tile([128, TB], F32, name=f"ps_o{o}",
                                          tag="acc") for o in range(6)]
                    for ck in range(KT):
                        st, sp_ = (ck == 0), (ck == KT - 1)
                        for h in range(NQ):
                            nc.tensor.matmul(
                                ps_o[h][:],
                                wq_t[ck][:, 128 * h:128 * (h + 1)],
                                xts[ck][:], start=st, stop=sp_)
                        nc.tensor.matmul(ps_o[4][:], wk_t[ck][:], xts[ck][:],
                                         start=st, stop=sp_)
                        nc.tensor.matmul(ps_o[5][:], wv_t[ck][:], xts[ck][:],
                                         start=st, stop=sp_)

                    # v: evict straight to vT, folding the int8 scale s_v
                    nc.vector.tensor_scalar(out=vT[:, js], in0=ps_o[5][:],
                                            scalar1=svb[0:128, 0:1],
                                            scalar2=None, op0=OP.mult)

                    # q heads and k: rmsnorm + rope
                    for o in range(5):
                        is_q = o < NQ
                        raw = tp.tile([128, TB], F32, name="raw", tag="raw",
                                      bufs=3)
                        nc.scalar.copy(raw[:], ps_o[o][:])
                        sq = tp.tile([128, TB], F32R, name="sq", tag="sq",
                                     bufs=2)
                        nc.vector.tensor_tensor(out=sq[:], in0=raw[:],
                                                in1=raw[:], op=OP.mult)
                        ps_r = psum_small.tile([1, TB], F32, name="ps_r",
                                               tag="small")
                        nc.tensor.matmul(ps_r[:], ones128[:], sq[:],
                                         start=True, stop=True)
                        rsq = tp.tile([1, TB], F32, name="rsq", tag="rsq",
                                      bufs=2)
                        nc.scalar.activation(rsq[:], ps_r[:], AF.Sqrt,
                                             bias=eps1[0:1, 0:1],
                                             scale=1.0 / HD)
                        rinv = tp.tile([1, TB], F32, name="rinv", tag="rinv",
                                       bufs=2)
                        nc.vector.reciprocal(rinv[:], rsq[:])
                        rsc = tp.tile([1, TB], F32R, name="rsc", tag="rsc",
                                      bufs=2)
                        if is_q:
                            nc.vector.tensor_scalar(
                                out=rsc[:], in0=rinv[:],
                                scalar1=qgain[0:1, o:o + 1], scalar2=None,
                                op0=OP.mult)
                        else:
                            nc.scalar.copy(rsc[:], rinv[:])
                        rb_s = tp.tile([128, TB], F32, name="rb_s",
                                       tag="rb_s", bufs=2)
                        nc.gpsimd.partition_broadcast(rb_s[:],
                                                      rsc[:].bitcast(F32))
                        # rope: rawsw = halves of raw swapped; sinb has -sin
                        # in its high half, so ro = raw*cos + rawsw*sin.
                        rawsw = tp.tile([128, TB], F32, name="rawsw",
                                        tag="rawsw", bufs=2)
                        nc.scalar.copy(rawsw[0:64, :], raw[64:128, :])
                        nc.scalar.copy(rawsw[64:128, :], raw[0:64, :])
                        rock = tp.tile([128, TB], F32, name="rock",
                                       tag="rock", bufs=2)
                        nc.vector.tensor_tensor(out=rock[:], in0=raw[:],
                                                in1=cosb[:, js], op=OP.mult)
                        rask = tp.tile([128, TB], F32, name="rask",
                                       tag="rask", bufs=2)
                        nc.vector.tensor_tensor(out=rask[:], in0=rawsw[:],
                                                in1=sinb[:, js], op=OP.mult)
                        ro = tp.tile([128, TB], F32, name="ro", tag="ro",
                                     bufs=2)
                        nc.vector.tensor_tensor(out=ro[:], in0=rock[:],
                                                in1=rask[:], op=OP.add)
                        dst = qf[o][:, js] if is_q else kf[:, js]
                        nc.vector.tensor_tensor(out=dst, in0=ro[:],
                                                in1=rb_s[:], op=OP.mult)

            # v transposed tiles [s, dh] for the attn@v matmul
            with tc.tile_pool(name="vtr", bufs=2) as vtrp:
                for i in range(ST):
                    ps_t = psum_acc.tile([128, 128], F32, name="ps_vt",
                                         tag="acc")
                    nc.tensor.transpose(ps_t[:], vT[:, 128 * i:128 * (i + 1)],
                                        identf[:])
                    nc.scalar.copy(vs[i][:], ps_t[:])

            # ---- SDPA + _xsa per t-block, then one AllGather + proj ----
            ybounce = dramp.tile([NQ * HD, T], BF16, name="ybounce")
            yfull = dramp.tile([4 * NQ * HD, T], BF16, name="yfull")

            with tc.tile_pool(name="sdpa", bufs=2) as sp:
                for j in range(NTB):
                    js = slice(TB * j, TB * (j + 1))
                    n_i = 4 * j + 4
                    denr = sp.tile([1, TB], F32, name="denr", tag="denr",
                                   bufs=2)
                    for h in range(NQ):
                        ps_y = psum_acc.tile([128, TB], F32, name="ps_y",
                                             tag="acc")
                        ps_z = psum_small.tile([1, TB], F32, name="ps_z",
                                               tag="small")
                        for i in range(n_i):
                            ps_s = psum_acc.tile([128, TB], F32, name="ps_s",
                                                 tag="acc")
                            nc.tensor.matmul(
                                ps_s[:], kf[:, 128 * i:128 * (i + 1)],
                                qf[h][:, js], start=True, stop=True)
                            if i >= 4 * j:
                                off = 128 * (i - 4 * j)
                                u0 = 384 - off
                                nc.vector.tensor_tensor(
                                    out=ps_s[:], in0=ps_s[:],
                                    in1=mask[:, u0:u0 + TB], op=OP.add)
                            et = sp.tile([128, TB], F32R, name="et",
                                         tag=f"et{i & 1}", bufs=2)
                            nc.scalar.activation(et[:], ps_s[:], AF.Exp,
                                                 scale=INV_SQRT_HD)
                            st, spp = (i == 0), (i == n_i - 1)
                            nc.tensor.matmul(ps_z[:], ones128[:], et[:],
                                             start=st, stop=spp,
                                             skip_group_check=True)
                            nc.tensor.matmul(ps_y[:], vs[i][:], et[:],
                                             start=st, stop=spp,
                                             skip_group_check=True)
                        # epilogue for (h, j)
                        y_h = sp.tile([128, TB], F32, name="y_h", tag="y_h",
                                      bufs=2)
                        nc.scalar.copy(y_h[:], ps_y[:])
                        if h == 0:
                            vsq = sp.tile([128, TB], F32R, name="vsq",
                                          tag="vsq", bufs=1)
                            nc.vector.tensor_tensor(out=vsq[:], in0=vT[:, js],
                                                    in1=vT[:, js],
                                                    op=OP.mult)
                            ps_d = psum_small.tile([1, TB], F32, name="ps_d",
                                                   tag="small")
                            nc.tensor.matmul(ps_d[:], ones128[:], vsq[:],
                                             start=True, stop=True)
                            den = sp.tile([1, TB], F32, name="den", tag="den",
                                          bufs=2)
                            nc.vector.tensor_scalar(out=den[:], in0=ps_d[:],
                                                    scalar1=1e-24,
                                                    scalar2=None, op0=OP.max)
                            nc.vector.reciprocal(denr[:], den[:])
                        zinv = sp.tile([1, TB], F32, name="zinv", tag="zinv",
                                       bufs=2)
                        nc.vector.reciprocal(zinv[:], ps_z[:])
                        zr = sp.tile([1, TB], F32R, name="zr", tag="zr",
                                     bufs=2)
                        nc.scalar.copy(zr[:], zinv[:])
                        yv = sp.tile([128, TB], F32R, name="yv", tag="yv",
                                     bufs=1)
                        nc.vector.tensor_tensor(out=yv[:], in0=y_h[:],
                                                in1=vT[:, js], op=OP.mult)
                        ps_dot = psum_small.tile([1, TB], F32, name="ps_dot",
                                                 tag="small")
                        nc.tensor.matmul(ps_dot[:], ones128[:], yv[:],
                                         start=True, stop=True)
                        c1 = sp.tile([1, TB], F32, name="c1", tag="c1",
                                     bufs=2)
                        nc.vector.tensor_tensor(out=c1[:], in0=ps_dot[:],
                                                in1=denr[:], op=OP.mult)
                        c2 = sp.tile([1, TB], F32R, name="c2", tag="c2",
                                     bufs=2)
                        nc.vector.tensor_tensor(out=c2[:], in0=c1[:],
                                                in1=zinv[:], op=OP.mult)
                        zb_s = sp.tile([128, TB], F32, name="zb_s",
                                       tag="zb_s", bufs=1)
                        cb_s = sp.tile([128, TB], F32, name="cb_s",
                                       tag="cb_s", bufs=1)
                        nc.gpsimd.partition_broadcast(zb_s[:],
                                                      zr[:].bitcast(F32))
                        nc.gpsimd.partition_broadcast(cb_s[:],
                                                      c2[:].bitcast(F32))
                        t1 = sp.tile([128, TB], F32, name="t1", tag="t1",
                                     bufs=1)
                        t2 = sp.tile([128, TB], F32, name="t2", tag="t2",
                                     bufs=1)
                        nc.vector.tensor_tensor(out=t1[:], in0=y_h[:],
                                                in1=zb_s[:], op=OP.mult)
                        nc.vector.tensor_tensor(out=t2[:], in0=vT[:, js],
                                                in1=cb_s[:], op=OP.mult)
                        yfin = sp.tile([128, TB], BF16, name="yfin",
                                       tag="yfin", bufs=2)
                        nc.vector.tensor_tensor(out=yfin[:], in0=t1[:],
                                                in1=t2[:], op=OP.subtract)
                        nc.sync.dma_start(
                            out=ybounce[128 * h:128 * (h + 1), js],
                            in_=yfin[:])
            nc.gpsimd.collective_compute(
                "AllGather", OP.bypass,
                replica_groups=[[0, 1, 2, 3], [4, 5, 6, 7]],
                ins=[ybounce[:].opt()], outs=[yfull[:].opt()])

            # ---- output projection (row-sharded: 512 out cols/core) ----
            # Accumulate the full f32 result in SBUF, then int8-quantize with
            # per-row scales (round-to-nearest + saturation on the convert).
            with tc.tile_pool(name="proj", bufs=2) as pp:
                ofull = [pp.tile([128, T], F32, name=f"ofull{o}",
                                 tag=f"ofull{o}", bufs=1) for o in range(4)]
                for j in range(NTB):
                    js = slice(TB * j, TB * (j + 1))
                    ps_p = [psum_acc.tile([128, TB], F32, name=f"ps_p{o}",
                                          tag="acc") for o in range(4)]
                    for ck in range(KT):
                        yt = pp.tile([128, TB], BF16, name="yt", tag="yt",
                                     bufs=4)
                        nc.sync.dma_start(
                            out=yt[:],
                            in_=yfull[128 * ck:128 * (ck + 1), js])
                        st, spp = (ck == 0), (ck == KT - 1)
                        for o in range(4):
                            nc.tensor.matmul(
                                ps_p[o][:],
                                wp_t[ck][:, 128 * o:128 * (o + 1)],
                                yt[:], start=st, stop=spp)
                    for o in range(4):
                        nc.vector.tensor_scalar(out=ofull[o][:, js],
                                                in0=ps_p[o][:],
                                                scalar1=spb[0:128, 0:1],
                                                scalar2=None, op0=OP.mult)
                # per-token absmax over all 512 out rows (partition all-
                # reduce per o-tile, then max across the 4 tiles)
                am = pp.tile([128, T], F32, name="am", tag="am", bufs=1)
                am2 = pp.tile([128, T], F32, name="am2", tag="am2", bufs=1)
                nc.gpsimd.partition_all_reduce(
                    am[:], ofull[0][:], channels=128,
                    reduce_op=bass_isa.ReduceOp.absmax)
                for o in range(1, 4):
                    nc.gpsimd.partition_all_reduce(
                        am2[:], ofull[o][:], channels=128,
                        reduce_op=bass_isa.ReduceOp.absmax)
                    nc.vector.tensor_tensor(out=am[:], in0=am[:],
                                            in1=am2[:], op=OP.max)
                nc.vector.tensor_scalar(out=am[:], in0=am[:], scalar1=1e-30,
                                        scalar2=None, op0=OP.max)
                osc = pp.tile([1, T], F32, name="osc", tag="osc", bufs=1)
                nc.vector.tensor_scalar(out=osc[:], in0=am[0:1, :],
                                        scalar1=1.0 / 127.0, scalar2=None,
                                        op0=OP.mult)
                nc.sync.dma_start(out=oscld[:], in_=osc[:])
                rsc = pp.tile([128, T], F32, name="rsc2", tag="rsc2", bufs=1)
                nc.vector.reciprocal(rsc[:], am[:])
                nc.vector.tensor_scalar(out=rsc[:], in0=rsc[:], scalar1=127.0,
                                        scalar2=None, op0=OP.mult)
                for o in range(4):
                    codes = pp.tile([128, T], I8, name="codes", tag="codes",
                                    bufs=2)
                    nc.vector.tensor_tensor(out=codes[:], in0=ofull[o][:],
                                            in1=rsc[:], op=OP.mult)
                    nc.sync.dma_start(out=outc[128 * o:128 * (o + 1), :],
                                      in_=codes[:])

    nc.compile()
    return nc


_NC = None


def _get_nc():
    global _NC
    if _NC is None:
        _NC = _build_nc()
    return _NC


_POOL = None


def _pool():
    global _POOL
    if _POOL is None:
        from concurrent.futures import ThreadPoolExecutor
        _POOL = ThreadPoolExecutor(max_workers=4)
    return _POOL


def _weight_codes(w, sf):
    """Host-side AnnealedBitLinear effective weight (f32, bitwise identical
    quantization decisions to the reference) + symmetric int8 encoding.
    Returns (codes int8, scale f32).  q/k scales never leave the host
    (rmsnorm makes q and k scale-invariant); s_v and s_p are folded back on
    device."""
    w = np.asarray(w, dtype=np.float32)
    wabs = np.abs(w)
    scale = np.clip(wabs.mean(axis=1, keepdims=True, dtype=np.float32),
                    1e-8, None).astype(np.float32)
    w_quant = np.where(wabs > 0.7 * scale,
                       np.copysign(scale, w).astype(np.float32),
                       np.float32(0.0))
    w_e = (1.0 - sf) * w + sf * w_quant
    s = np.float32(max(np.abs(w_e).max() / 127.0, 1e-30))
    codes = np.clip(np.rint(w_e * np.float32(1.0 / s)), -127, 127) \
        .astype(np.int8)
    return codes, s


def _make_in_maps(x, step_fraction, w_q, w_k, w_v, w_proj, q_gain):
    x = np.asarray(x, dtype=np.float32)
    sf = np.float32(np.asarray(step_fraction, dtype=np.float32).reshape(-1)[0])
    q_gain = np.asarray(q_gain, dtype=np.float32)
    futs = [_pool().submit(_weight_codes, w, sf)
            for w in (w_q, w_k, w_v, w_proj)]
    # overlap the x slice+cast work with the weight quantization
    def _xq(b, h):
        return np.ascontiguousarray(
            x[b][:, 512 * h:512 * (h + 1)]).astype(NPBF16)
    xq_futs = [_pool().submit(_xq, c // 4, c % 4) for c in range(N_CORES)]
    (wq_c, _), (wk_c, _), (wv_c, s_v), (wp_c, s_p) = \
        [f.result() for f in futs]
    scl = np.array([[s_v, s_p]], dtype=np.float32)
    # packed per-head-group weight code blocks [1280, D] int8
    wpacks = []
    for h in range(4):
        wpacks.append(np.concatenate([
            wq_c[512 * h:512 * (h + 1), :],
            wk_c[128 * h:128 * (h + 1), :],
            wv_c[128 * h:128 * (h + 1), :],
            wp_c[512 * h:512 * (h + 1), :]], axis=0))
    in_maps = []
    half = WROWS // 2
    for c in range(N_CORES):
        b, h = divmod(c, 4)
        in_maps.append({
            "xq": xq_futs[c].result(),
            "wpack": np.ascontiguousarray(
                wpacks[h][half * b:half * (b + 1), :]),
            "qgain": np.ascontiguousarray(q_gain[4 * h:4 * (h + 1)]
                                          .reshape(1, NQ)),
            "scl": scl,
        })
    return in_maps


def _assemble(results):
    out = np.empty((2, T, D), dtype=np.float32)

    def _decode(c):
        b, h = divmod(c, 4)
        codes = results[c]["outc"]            # [512, T] int8
        scl = results[c]["oscl"]              # [1, T] f32 per-token scale
        dec = np.multiply(codes, scl, dtype=np.float32)
        out[b][:, 512 * h:512 * (h + 1)] = dec.T

    list(_pool().map(_decode, range(N_CORES)))
    return out


def kernel(**inputs) -> np.ndarray:
    nc = _get_nc()
    in_maps = _make_in_maps(**inputs)
    res = bass_utils.run_bass_kernel_spmd(nc, in_maps,
                                          core_ids=list(range(N_CORES)))
    return _assemble(res.results)


def bench(**inputs):
    """Returns (output, BassKernelResults); tracing if the env supports it."""
    nc = _get_nc()
    in_maps = _make_in_maps(**inputs)
    try:
        res = bass_utils.run_bass_kernel_spmd(nc, in_maps,
                                              core_ids=list(range(N_CORES)),
                                              trace=True)
    except ModuleNotFoundError:
        res = bass_utils.run_bass_kernel_spmd(nc, in_maps,
                                              core_ids=list(range(N_CORES)))
    return _assemble(res.results), res


# revision 15
# speedup vs baseline: 1.0488x; 1.0376x over previous
"""Trainium2 Bass kernel for nn_CausalSelfAttention_60284160967096 (v4).

Sharding: 8 cores = 2 (batch) x 4 (kv-head groups).  Each core computes its
batch's attention for one kv-head (4 query heads), the Gram-Schmidt (_xsa)
correction, then an AllGather of y within the 4-core group and a row-sharded
output projection producing a 512-column slice of the output.

The axon tunnel (~44 MB/s) dominates wall time, so wire bytes are minimized:
  - x / weights / output cross the wire in bf16
  - x is shipped as per-core feature quarters in natural [T, 512] layout
    (16 MB total instead of 4x-duplicated 128 MB) and AllGathered on-device
    after an on-device PE transpose
  - weight slices are packed into one [1280, D] block per head group; each
    core ships HALF of it and an AllGather over (b=0,b=1) pairs rebuilds the
    full block (21 MB instead of 80 MB)
  - rope tables / causal mask / identity are inline_tensor NEFF constants
    (zero wire bytes)
The ternary-quantized *effective* weights are computed on the host in f32
(bitwise-identical quantization decisions to the reference; device-side
quantization of bf16-rounded weights flips ~0.14% of ternary decisions and
costs ~2% rel error).  SDPA/rmsnorm/rope/_xsa stay fp32/fp32r on device;
only wire-adjacent tensors are bf16.
"""

import numpy as np

import jax

import concourse.bass as bass
import concourse.bass_isa as bass_isa
import concourse.mybir as mybir
import concourse.tile as tile
from concourse import bacc, bass_utils

# Persistent XLA compilation cache: run_bass_kernel_spmd builds a fresh jit
# closure per call, which otherwise re-pays ~1s of XLA compile every call.
try:
    jax.config.update("jax_compilation_cache_dir", "/tmp/jax_cc_nnattn")
    jax.config.update("jax_persistent_cache_min_compile_time_secs", 0.0)
    jax.config.update("jax_persistent_cache_min_entry_size_bytes", 0)
except Exception:
    pass

F32 = mybir.dt.float32
F32R = mybir.dt.float32r
BF16 = mybir.dt.bfloat16
FP16 = mybir.dt.float16
I8 = mybir.dt.int8
NPBF16 = mybir.dt.np(BF16)
NPFP16 = mybir.dt.np(FP16)
AF = mybir.ActivationFunctionType
OP = mybir.AluOpType

T = 2048
D = 2048
HD = 128
NQ = 4          # query heads per core
TB = 512        # token block
NTB = T // TB   # 4
KT = D // 128   # 16 contraction tiles
ST = T // 128   # 16 s tiles
N_CORES = 8
WROWS = NQ * HD + HD + HD + NQ * HD   # 1280 packed weight rows
RMS_EPS = 1.1920928955078125e-07
INV_SQRT_HD = float(np.float32(1.0) / np.sqrt(np.float32(HD)))
NEG_BIG = -1.0e30


def _host_constants():
    t = np.arange(T, dtype=np.float32)
    inv_freq = (1.0 / 10000.0 ** (np.arange(0, HD, 2, dtype=np.float32) / HD))
    freqs = np.outer(t, inv_freq).astype(np.float32)        # [T, 64]
    cos_h = np.cos(freqs).T.astype(np.float32)              # [64, T]
    sin_h = np.sin(freqs).T.astype(np.float32)
    cosT = np.ascontiguousarray(np.concatenate([cos_h, cos_h], axis=0))
    sinT = np.ascontiguousarray(np.concatenate([sin_h, -sin_h], axis=0))
    s = np.arange(128)[:, None]
    u = np.arange(896)[None, :]
    maskadd = np.where(u >= s + 384, 0.0, NEG_BIG).astype(np.float32)
    ident = np.eye(128, dtype=np.float32)
    return cosT, sinT, maskadd, ident


def _build_nc():
    nc = bacc.Bacc("TRN2", target_bir_lowering=False, debug=False,
                   num_devices=N_CORES)

    # per-core external inputs: x bf16, weights int8 codes + fold scales
    xqd = nc.dram_tensor("xq", [T, 512], BF16, kind="ExternalInput")
    wpd = nc.dram_tensor("wpack", [WROWS // 2, D], I8, kind="ExternalInput")
    qgaind = nc.dram_tensor("qgain", [1, NQ], F32, kind="ExternalInput")
    scld = nc.dram_tensor("scl", [1, 2], F32, kind="ExternalInput")
    # int8 output codes + per-token f32 scales (halves fetch + zero-buffer
    # wire; the output is heavy-tailed per row, so scales go per token)
    outc = nc.dram_tensor("outc", [NQ * HD, T], I8, kind="ExternalOutput")
    oscld = nc.dram_tensor("oscl", [1, T], F32, kind="ExternalOutput")

    # NEFF-embedded rope tables (no wire bytes; fp16 keeps the BIR small,
    # rounding is ~2e-4).  Mask and identities are generated on device.
    cos_np, sin_np, _mask_np, _ident_np = _host_constants()
    cosd = nc.inline_tensor(cos_np.astype(NPFP16), name="cosT")
    sind = nc.inline_tensor(sin_np.astype(NPFP16), name="sinT")

    with nc.allow_low_precision(reason="bf16 wire + fp32r matmul pipeline"), \
         tile.TileContext(nc) as tc:
        with (
            tc.tile_pool(name="const", bufs=1) as constp,
            tc.tile_pool(name="acts", bufs=1) as actp,
            tc.tile_pool(name="weights", bufs=1) as wp,
            tc.tile_pool(name="psum_acc", bufs=6, space="PSUM") as psum_acc,
            tc.tile_pool(name="psum_small", bufs=2, space="PSUM") as psum_small,
            tc.tile_pool(name="dram", bufs=1, space="DRAM") as dramp,
        ):
            # ---- constants ----
            onesf = constp.tile([128, 1], F32)
            nc.vector.memset(onesf[:], 1.0)
            ones128 = constp.tile([128, 1], F32R)
            nc.scalar.copy(ones128[:], onesf[:])
            # causal mask: keep 0 where u >= s + 384, else NEG_BIG
            mask = constp.tile([128, 896], F32)
            nc.gpsimd.memset(mask[:], 0.0)
            nc.gpsimd.affine_select(out=mask[:], in_=mask[:],
                                    pattern=[[1, 896]], base=-384,
                                    channel_multiplier=-1,
                                    compare_op=OP.is_ge, fill=NEG_BIG)
            cosh = constp.tile([HD, T], FP16)
            nc.sync.dma_start(out=cosh[:], in_=cosd[:])
            cosb = constp.tile([HD, T], F32)
            nc.vector.tensor_copy(cosb[:], cosh[:])
            sinh = constp.tile([HD, T], FP16)
            nc.sync.dma_start(out=sinh[:], in_=sind[:])
            sinb = constp.tile([HD, T], F32)
            nc.vector.tensor_copy(sinb[:], sinh[:])
            # identities (transpose operands): diag(1) via affine_select
            onesb2 = constp.tile([128, 128], BF16)
            nc.vector.memset(onesb2[:], 1.0)
            identb = constp.tile([128, 128], BF16)
            nc.gpsimd.affine_select(out=identb[:], in_=onesb2[:],
                                    pattern=[[1, 128]], base=0,
                                    channel_multiplier=-1,
                                    compare_op=OP.is_equal, fill=0.0)
            onesf2 = constp.tile([128, 128], F32)
            nc.vector.memset(onesf2[:], 1.0)
            identf = constp.tile([128, 128], F32)
            nc.gpsimd.affine_select(out=identf[:], in_=onesf2[:],
                                    pattern=[[1, 128]], base=0,
                                    channel_multiplier=-1,
                                    compare_op=OP.is_equal, fill=0.0)
            qgain = constp.tile([1, NQ], F32)
            nc.sync.dma_start(out=qgain[:], in_=qgaind[:])
            scl = constp.tile([1, 2], F32)
            nc.sync.dma_start(out=scl[:], in_=scld[:])
            svb = constp.tile([128, 1], F32)
            nc.gpsimd.partition_broadcast(svb[:], scl[0:1, 0:1])
            spb = constp.tile([128, 1], F32)
            nc.gpsimd.partition_broadcast(spb[:], scl[0:1, 1:2])
            eps1 = constp.tile([1, 1], F32)
            nc.vector.memset(eps1[:], RMS_EPS)

            # ---- weight AllGather across the (b=0, b=1) pair ----
            wtb = dramp.tile([WROWS // 2, D], I8, name="wtb")
            wfull = dramp.tile([WROWS, D], I8, name="wfull")
            nc.sync.dma_start(out=wtb[:], in_=wpd[:])
            nc.gpsimd.collective_compute(
                "AllGather", OP.bypass,
                replica_groups=[[0, 4], [1, 5], [2, 6], [3, 7]],
                ins=[wtb[:].opt()], outs=[wfull[:].opt()])

            # ---- x transpose (on-device) + AllGather across head groups ----
            xtb = dramp.tile([512, T], BF16, name="xtb")
            xTd = dramp.tile([D, T], BF16, name="xTd")
            with tc.tile_pool(name="xtr", bufs=1) as xtrp:
                xTq = [xtrp.tile([128, T], BF16, name=f"xTq{fc}",
                                 tag=f"xTq{fc}") for fc in range(4)]
                for tr in range(ST):
                    xt_in = xtrp.tile([128, 512], BF16, name="xt_in",
                                      tag="xt_in", bufs=4)
                    nc.sync.dma_start(out=xt_in[:],
                                      in_=xqd[128 * tr:128 * (tr + 1), :])
                    for fc in range(4):
                        ps_t = psum_acc.tile([128, 128], BF16, name="ps_xt",
                                             tag="acc")
                        nc.tensor.transpose(
                            ps_t[:], xt_in[:, 128 * fc:128 * (fc + 1)],
                            identb[:])
                        nc.vector.tensor_copy(
                            xTq[fc][:, 128 * tr:128 * (tr + 1)], ps_t[:])
                for fc in range(4):
                    nc.sync.dma_start(out=xtb[128 * fc:128 * (fc + 1), :],
                                      in_=xTq[fc][:])
            nc.gpsimd.collective_compute(
                "AllGather", OP.bypass,
                replica_groups=[[0, 1, 2, 3], [4, 5, 6, 7]],
                ins=[xtb[:].opt()], outs=[xTd[:].opt()])

            # ---- weight transpose straight into effective weight tiles ----
            # wfull rows: q 0:512, k 512:640, v 640:768, p 768:1280
            wq_t = [wp.tile([128, NQ * HD], BF16, name=f"wq{ck}",
                            tag=f"wq{ck}") for ck in range(KT)]
            wk_t = [wp.tile([128, HD], BF16, name=f"wk{ck}", tag=f"wk{ck}")
                    for ck in range(KT)]
            wv_t = [wp.tile([128, HD], BF16, name=f"wv{ck}", tag=f"wv{ck}")
                    for ck in range(KT)]
            wp_t = [wp.tile([128, NQ * HD], BF16, name=f"wpj{ck}",
                            tag=f"wpj{ck}") for ck in range(KT)]
            with tc.tile_pool(name="wtr", bufs=1) as wtrp:
                for rt in range(10):
                    w_i8 = wtrp.tile([128, D], I8, name="w_i8",
                                     tag="w_i8", bufs=3)
                    nc.sync.dma_start(
                        out=w_i8[:], in_=wfull[128 * rt:128 * (rt + 1), :])
                    w_in = wtrp.tile([128, D], BF16, name="w_in",
                                     tag="w_in", bufs=3)
                    nc.vector.tensor_copy(w_in[:], w_i8[:])
                    if rt < 4:
                        dst, r = wq_t, rt
                    elif rt == 4:
                        dst, r = wk_t, 0
                    elif rt == 5:
                        dst, r = wv_t, 0
                    else:
                        dst, r = wp_t, rt - 6
                    for ck in range(KT):
                        ps_t = psum_acc.tile([128, 128], BF16, name="ps_wt",
                                             tag="acc")
                        nc.tensor.transpose(
                            ps_t[:], w_in[:, 128 * ck:128 * (ck + 1)],
                            identb[:])
                        nc.vector.tensor_copy(
                            dst[ck][:, 128 * r:128 * (r + 1)], ps_t[:])

            # ---- persistent activations ----
            qf = [actp.tile([128, T], F32R, name=f"qf{h}", tag=f"qf{h}")
                  for h in range(NQ)]
            kf = actp.tile([128, T], F32R, name="kf", tag="kf")
            vT = actp.tile([128, T], F32, name="vT", tag="vT")
            vs = [actp.tile([128, 128], F32R, name=f"vs{i}", tag=f"vs{i}")
                  for i in range(ST)]

            # ---- QKV projections + rmsnorm + rope ----
            with tc.tile_pool(name="qkv_tmp", bufs=2) as tp:
                for j in range(NTB):
                    js = slice(TB * j, TB * (j + 1))
                    # stream x k-tiles for this t-block from xTd (bf16)
                    xts = []
                    for ck in range(KT):
                        xt = tp.tile([128, TB], BF16, name="xt",
                                     tag="xt", bufs=4)
                        nc.sync.dma_start(
                            out=xt[:], in_=xTd[128 * ck:128 * (ck + 1), js])
                        xts.append(xt)
                    ps_o = [psum_acc.tile([128, TB], F32, name=f"ps_o{o}",
                                          tag="acc") for o in range(6)]
                    for ck in range(KT):
                        st, sp_ = (ck == 0), (ck == KT - 1)
                        for h in range(NQ):
                            nc.tensor.matmul(
                                ps_o[h][:],
                                wq_t[ck][:, 128 * h:128 * (h + 1)],
                                xts[ck][:], start=st, stop=sp_)
                        nc.tensor.matmul(ps_o[4][:], wk_t[ck][:], xts[ck][:],
                                         start=st, stop=sp_)
                        nc.tensor.matmul(ps_o[5][:], wv_t[ck][:], xts[ck][:],
                                         start=st, stop=sp_)

                    # v: evict straight to vT, folding the int8 scale s_v
                    nc.vector.tensor_scalar(out=vT[:, js], in0=ps_o[5][:],
                                            scalar1=svb[0:128, 0:1],
                                            scalar2=None, op0=OP.mult)

                    # q heads and k: rmsnorm + rope
                    for o in range(5):
                        is_q = o < NQ
                        raw = tp.tile([128, TB], F32, name="raw", tag="raw",
                                      bufs=3)
                        nc.scalar.copy(raw[:], ps_o[o][:])
                        sq = tp.tile([128, TB], F32R, name="sq", tag="sq",
                                     bufs=2)
                        nc.vector.tensor_tensor(out=sq[:], in0=raw[:],
                                                in1=raw[:], op=OP.mult)
                        ps_r = psum_small.tile([1, TB], F32, name="ps_r",
                                               tag="small")
                        nc.tensor.matmul(ps_r[:], ones128[:], sq[:],
                                         start=True, stop=True)
                        rsq = tp.tile([1, TB], F32, name="rsq", tag="rsq",
                                      bufs=2)
                        nc.scalar.activation(rsq[:], ps_r[:], AF.Sqrt,
                                             bias=eps1[0:1, 0:1],
                                             scale=1.0 / HD)
                        rinv = tp.tile([1, TB], F32, name="rinv", tag="rinv",
                                       bufs=2)
                        nc.vector.reciprocal(rinv[:], rsq[:])
                        rsc = tp.tile([1, TB], F32R, name="rsc", tag="rsc",
                                      bufs=2)
                        if is_q:
                            nc.vector.tensor_scalar(
                                out=rsc[:], in0=rinv[:],
                                scalar1=qgain[0:1, o:o + 1], scalar2=None,
                                op0=OP.mult)
                        else:
                            nc.scalar.copy(rsc[:], rinv[:])
                        rb_s = tp.tile([128, TB], F32, name="rb_s",
                                       tag="rb_s", bufs=2)
                        nc.gpsimd.partition_broadcast(rb_s[:],
                                                      rsc[:].bitcast(F32))
                        # rope: rawsw = halves of raw swapped; sinb has -sin
                        # in its high half, so ro = raw*cos + rawsw*sin.
                        rawsw = tp.tile([128, TB], F32, name="rawsw",
                                        tag="rawsw", bufs=2)
                        nc.scalar.copy(rawsw[0:64, :], raw[64:128, :])
                        nc.scalar.copy(rawsw[64:128, :], raw[0:64, :])
                        rock = tp.tile([128, TB], F32, name="rock",
                                       tag="rock", bufs=2)
                        nc.vector.tensor_tensor(out=rock[:], in0=raw[:],
                                                in1=cosb[:, js], op=OP.mult)
                        rask = tp.tile([128, TB], F32, name="rask",
                                       tag="rask", bufs=2)
                        nc.vector.tensor_tensor(out=rask[:], in0=rawsw[:],
                                                in1=sinb[:, js], op=OP.mult)
                        ro = tp.tile([128, TB], F32, name="ro", tag="ro",
                                     bufs=2)
                        nc.vector.tensor_tensor(out=ro[:], in0=rock[:],
                                                in1=rask[:], op=OP.add)
                        dst = qf[o][:, js] if is_q else kf[:, js]
                        nc.vector.tensor_tensor(out=dst, in0=ro[:],
                                                in1=rb_s[:], op=OP.mult)

            # v transposed tiles [s, dh] for the attn@v matmul
            with tc.tile_pool(name="vtr", bufs=2) as vtrp:
                for i in range(ST):
                    ps_t = psum_acc.tile([128, 128], F32, name="ps_vt",
                                         tag="acc")
                    nc.tensor.transpose(ps_t[:], vT[:, 128 * i:128 * (i + 1)],
                                        identf[:])
                    nc.scalar.copy(vs[i][:], ps_t[:])

            # ---- SDPA + _xsa per t-block, then one AllGather + proj ----
            ybounce = dramp.tile([NQ * HD, T], BF16, name="ybounce")
            yfull = dramp.tile([4 * NQ * HD, T], BF16, name="yfull")

            with tc.tile_pool(name="sdpa", bufs=2) as sp:
                for j in range(NTB):
                    js = slice(TB * j, TB * (j + 1))
                    n_i = 4 * j + 4
                    denr = sp.tile([1, TB], F32, name="denr", tag="denr",
                                   bufs=2)
                    for h in range(NQ):
                        ps_y = psum_acc.tile([128, TB], F32, name="ps_y",
                                             tag="acc")
                        ps_z = psum_small.tile([1, TB], F32, name="ps_z",
                                               tag="small")
                        for i in range(n_i):
                            ps_s = psum_acc.tile([128, TB], F32, name="ps_s",
                                                 tag="acc")
                            nc.tensor.matmul(
                                ps_s[:], kf[:, 128 * i:128 * (i + 1)],
                                qf[h][:, js], start=True, stop=True)
                            if i >= 4 * j:
                                off = 128 * (i - 4 * j)
                                u0 = 384 - off
                                nc.vector.tensor_tensor(
                                    out=ps_s[:], in0=ps_s[:],
                                    in1=mask[:, u0:u0 + TB], op=OP.add)
                            et = sp.tile([128, TB], F32R, name="et",
                                         tag=f"et{i & 1}", bufs=2)
                            nc.scalar.activation(et[:], ps_s[:], AF.Exp,
                                                 scale=INV_SQRT_HD)
                            st, spp = (i == 0), (i == n_i - 1)
                            nc.tensor.matmul(ps_z[:], ones128[:], et[:],
                                             start=st, stop=spp,
                                             skip_group_check=True)
                            nc.tensor.matmul(ps_y[:], vs[i][:], et[:],
                                             start=st, stop=spp,
                                             skip_group_check=True)
                        # epilogue for (h, j)
                        y_h = sp.tile([128, TB], F32, name="y_h", tag="y_h",
                                      bufs=2)
                        nc.scalar.copy(y_h[:], ps_y[:])
                        if h == 0:
                            vsq = sp.tile([128, TB], F32R, name="vsq",
                                          tag="vsq", bufs=1)
                            nc.vector.tensor_tensor(out=vsq[:], in0=vT[:, js],
                                                    in1=vT[:, js],
                                                    op=OP.mult)
                            ps_d = psum_small.tile([1, TB], F32, name="ps_d",
                                                   tag="small")
                            nc.tensor.matmul(ps_d[:], ones128[:], vsq[:],
                                             start=True, stop=True)
                            den = sp.tile([1, TB], F32, name="den", tag="den",
                                          bufs=2)
                            nc.vector.tensor_scalar(out=den[:], in0=ps_d[:],
                                                    scalar1=1e-24,
                                                    scalar2=None, op0=OP.max)
                            nc.vector.reciprocal(denr[:], den[:])
                        zinv = sp.tile([1, TB], F32, name="zinv", tag="zinv",
                                       bufs=2)
                        nc.vector.reciprocal(zinv[:], ps_z[:])
                        zr = sp.tile([1, TB], F32R, name="zr", tag="zr",
                                     bufs=2)
                        nc.scalar.copy(zr[:], zinv[:])
                        yv = sp.tile([128, TB], F32R, name="yv", tag="yv",
                                     bufs=1)
                        nc.vector.tensor_tensor(out=yv[:], in0=y_h[:],
                                                in1=vT[:, js], op=OP.mult)
                        ps_dot = psum_small.tile([1, TB], F32, name="ps_dot",
                                                 tag="small")
                        nc.tensor.matmul(ps_dot[:], ones128[:], yv[:],
                                         start=True, stop=True)
                        c1 = sp.tile([1, TB], F32, name="c1", tag="c1",
                                     bufs=2)
                        nc.vector.tensor_tensor(out=c1[:], in0=ps_dot[:],
                                                in1=denr[:], op=OP.mult)
                        c2 = sp.tile([1, TB], F32R, name="c2", tag="c2",
                                     bufs=2)
                        nc.vector.tensor_tensor(out=c2[:], in0=c1[:],
                                                in1=zinv[:], op=OP.mult)
                        zb_s = sp.tile([128, TB], F32, name="zb_s",
                                       tag="zb_s", bufs=1)
                        cb_s = sp.tile([128, TB], F32, name="cb_s",
                                       tag="cb_s", bufs=1)
                        nc.gpsimd.partition_broadcast(zb_s[:],
                                                      zr[:].bitcast(F32))
                        nc.gpsimd.partition_broadcast(cb_s[:],
                                                      c2[:].bitcast(F32))
                        t1 = sp.tile([128, TB], F32, name="t1", tag="t1",
                                     bufs=1)
                        t2 = sp.tile([128, TB], F32, name="t2", tag="t2",
                                     bufs=1)
                        nc.vector.tensor_tensor(out=t1[:], in0=y_h[:],
                                                in1=zb_s[:], op=OP.mult)
                        nc.vector.tensor_tensor(out=t2[:], in0=vT[:, js],
                                                in1=cb_s[:], op=OP.mult)
                        yfin = sp.tile([128, TB], BF16, name="yfin",
                                       tag="yfin", bufs=2)
                        nc.vector.tensor_tensor(out=yfin[:], in0=t1[:],
                                                in1=t2[:], op=OP.subtract)
                        nc.sync.dma_start(
                            out=ybounce[128 * h:128 * (h + 1), js],
                            in_=yfin[:])
            nc.gpsimd.collective_compute(
                "AllGather", OP.bypass,
                replica_groups=[[0, 1, 2, 3], [4, 5, 6, 7]],
                ins=[ybounce[:].opt()], outs=[yfull[:].opt()])

            # ---- output projection (row-sharded: 512 out cols/core) ----
            # Accumulate the full f32 result in SBUF, then int8-quantize with
            # per-row scales (round-to-nearest + saturation on the convert).
            with tc.tile_pool(name="proj", bufs=2) as pp:
                ofull = [pp.tile([128, T], F32, name=f"ofull{o}",
                                 tag=f"ofull{o}", bufs=1) for o in range(4)]
                for j in range(NTB):
                    js = slice(TB * j, TB * (j + 1))
                    ps_p = [psum_acc.tile([128, TB], F32, name=f"ps_p{o}",
                                          tag="acc") for o in range(4)]
                    for ck in range(KT):
                        yt = pp.tile([128, TB], BF16, name="yt", tag="yt",
                                     bufs=4)
                        nc.sync.dma_start(
                            out=yt[:],
                            in_=yfull[128 * ck:128 * (ck + 1), js])
                        st, spp = (ck == 0), (ck == KT - 1)
                        for o in range(4):
                            nc.tensor.matmul(
                                ps_p[o][:],
                                wp_t[ck][:, 128 * o:128 * (o + 1)],
                                yt[:], start=st, stop=spp)
                    for o in range(4):
                        nc.vector.tensor_scalar(out=ofull[o][:, js],
                                                in0=ps_p[o][:],
                                                scalar1=spb[0:128, 0:1],
                                                scalar2=None, op0=OP.mult)
                # per-token absmax over all 512 out rows (partition all-
                # reduce per o-tile, then max across the 4 tiles)
                am = pp.tile([128, T], F32, name="am", tag="am", bufs=1)
                am2 = pp.tile([128, T], F32, name="am2", tag="am2", bufs=1)
                nc.gpsimd.partition_all_reduce(
                    am[:], ofull[0][:], channels=128,
                    reduce_op=bass_isa.ReduceOp.absmax)
                for o in range(1, 4):
                    nc.gpsimd.partition_all_reduce(
                        am2[:], ofull[o][:], channels=128,
                        reduce_op=bass_isa.ReduceOp.absmax)
                    nc.vector.tensor_tensor(out=am[:], in0=am[:],
                                            in1=am2[:], op=OP.max)
                nc.vector.tensor_scalar(out=am[:], in0=am[:], scalar1=1e-30,
                                        scalar2=None, op0=OP.max)
                osc = pp.tile([1, T], F32, name="osc", tag="osc", bufs=1)
                nc.vector.tensor_scalar(out=osc[:], in0=am[0:1, :],
                                        scalar1=1.0 / 127.0, scalar2=None,
                                        op0=OP.mult)
                nc.sync.dma_start(out=oscld[:], in_=osc[:])
                rsc = pp.tile([128, T], F32, name="rsc2", tag="rsc2", bufs=1)
                nc.vector.reciprocal(rsc[:], am[:])
                nc.vector.tensor_scalar(out=rsc[:], in0=rsc[:], scalar1=127.0,
                                        scalar2=None, op0=OP.mult)
                for o in range(4):
                    codes = pp.tile([128, T], I8, name="codes", tag="codes",
                                    bufs=2)
                    nc.vector.tensor_tensor(out=codes[:], in0=ofull[o][:],
                                            in1=rsc[:], op=OP.mult)
                    nc.sync.dma_start(out=outc[128 * o:128 * (o + 1), :],
                                      in_=codes[:])

    nc.compile()
    return nc


_NC = None


def _get_nc():
    global _NC
    if _NC is None:
        _NC = _build_nc()
    return _NC


_POOL = None


def _pool():
    global _POOL
    if _POOL is None:
        from concurrent.futures import ThreadPoolExecutor
        _POOL = ThreadPoolExecutor(max_workers=4)
    return _POOL


def _weight_codes(w, sf):
    """Host-side AnnealedBitLinear effective weight (f32, bitwise identical
    quantization decisions to the reference) + symmetric int8 encoding.
    Returns (codes int8, scale f32).  q/k scales never leave the host
    (rmsnorm makes q and k scale-invariant); s_v and s_p are folded back on
    device."""
    w = np.asarray(w, dtype=np.float32)
    wabs = np.abs(w)
    scale = np.clip(wabs.mean(axis=1, keepdims=True, dtype=np.float32),
                    1e-8, None).astype(np.float32)
    cond = wabs > (np.float32(0.7) * scale)
    # w_e = (1-sf)*w everywhere, += sf*copysign(scale, w) where cond
    # (identical quantization decisions; wabs buffer reused for w_e)
    w_e = np.multiply(w, np.float32(1.0 - sf), out=wabs)
    np.add(w_e, np.copysign(scale * sf, w), out=w_e, where=cond)
    # exact absmax without materializing |w_e|; scale maps max to +/-127
    # exactly, so no clip is needed before the int8 cast
    s = np.float32(max(max(w_e.max(), -w_e.min()) / 127.0, 1e-30))
    np.multiply(w_e, np.float32(1.0 / s), out=w_e)
    np.rint(w_e, out=w_e)
    return w_e.astype(np.int8), s


def _make_in_maps(x, step_fraction, w_q, w_k, w_v, w_proj, q_gain):
    x = np.asarray(x, dtype=np.float32)
    sf = np.float32(np.asarray(step_fraction, dtype=np.float32).reshape(-1)[0])
    q_gain = np.asarray(q_gain, dtype=np.float32)
    futs = [_pool().submit(_weight_codes, w, sf)
            for w in (w_q, w_k, w_v, w_proj)]
    # overlap the x slice+cast work with the weight quantization
    def _xq(b, h):
        return np.ascontiguousarray(
            x[b][:, 512 * h:512 * (h + 1)]).astype(NPBF16)
    xq_futs = [_pool().submit(_xq, c // 4, c % 4) for c in range(N_CORES)]
    (wq_c, _), (wk_c, _), (wv_c, s_v), (wp_c, s_p) = \
        [f.result() for f in futs]
    scl = np.array([[s_v, s_p]], dtype=np.float32)
    # packed per-head-group weight code blocks [1280, D] int8
    wpacks = []
    for h in range(4):
        wpacks.append(np.concatenate([
            wq_c[512 * h:512 * (h + 1), :],
            wk_c[128 * h:128 * (h + 1), :],
            wv_c[128 * h:128 * (h + 1), :],
            wp_c[512 * h:512 * (h + 1), :]], axis=0))
    in_maps = []
    half = WROWS // 2
    for c in range(N_CORES):
        b, h = divmod(c, 4)
        in_maps.append({
            "xq": xq_futs[c].result(),
            "wpack": np.ascontiguousarray(
                wpacks[h][half * b:half * (b + 1), :]),
            "qgain": np.ascontiguousarray(q_gain[4 * h:4 * (h + 1)]
                                          .reshape(1, NQ)),
            "scl": scl,
        })
    return in_maps


def _assemble(results):
    out = np.empty((2, T, D), dtype=np.float32)

    def _decode(c):
        b, h = divmod(c, 4)
        codes = results[c]["outc"]            # [512, T] int8
        scl = results[c]["oscl"]              # [1, T] f32 per-token scale
        dec = np.multiply(codes, scl, dtype=np.float32)
        out[b][:, 512 * h:512 * (h + 1)] = dec.T

    list(_pool().map(_decode, range(N_CORES)))
    return out


def kernel(**inputs) -> np.ndarray:
    nc = _get_nc()
    in_maps = _make_in_maps(**inputs)
    res = bass_utils.run_bass_kernel_spmd(nc, in_maps,
                                          core_ids=list(range(N_CORES)))
    return _assemble(res.results)


def bench(**inputs):
    """Returns (output, BassKernelResults); tracing if the env supports it."""
    nc = _get_nc()
    in_maps = _make_in_maps(**inputs)
    try:
        res = bass_utils.run_bass_kernel_spmd(nc, in_maps,
                                              core_ids=list(range(N_CORES)),
                                              trace=True)
    except ModuleNotFoundError:
        res = bass_utils.run_bass_kernel_spmd(nc, in_maps,
                                              core_ids=list(range(N_CORES)))
    return _assemble(res.results), res


# revision 16
# speedup vs baseline: 1.1062x; 1.0547x over previous
"""Trainium2 Bass kernel for nn_CausalSelfAttention_60284160967096 (v4).

Sharding: 8 cores = 2 (batch) x 4 (kv-head groups).  Each core computes its
batch's attention for one kv-head (4 query heads), the Gram-Schmidt (_xsa)
correction, then an AllGather of y within the 4-core group and a row-sharded
output projection producing a 512-column slice of the output.

The axon tunnel (~44 MB/s) dominates wall time, so wire bytes are minimized:
  - x / weights / output cross the wire in bf16
  - x is shipped as per-core feature quarters in natural [T, 512] layout
    (16 MB total instead of 4x-duplicated 128 MB) and AllGathered on-device
    after an on-device PE transpose
  - weight slices are packed into one [1280, D] block per head group; each
    core ships HALF of it and an AllGather over (b=0,b=1) pairs rebuilds the
    full block (21 MB instead of 80 MB)
  - rope tables / causal mask / identity are inline_tensor NEFF constants
    (zero wire bytes)
The ternary-quantized *effective* weights are computed on the host in f32
(bitwise-identical quantization decisions to the reference; device-side
quantization of bf16-rounded weights flips ~0.14% of ternary decisions and
costs ~2% rel error).  SDPA/rmsnorm/rope/_xsa stay fp32/fp32r on device;
only wire-adjacent tensors are bf16.
"""

import numpy as np

import jax

import concourse.bass as bass
import concourse.bass_isa as bass_isa
import concourse.mybir as mybir
import concourse.tile as tile
from concourse import bacc, bass_utils

# Persistent XLA compilation cache: run_bass_kernel_spmd builds a fresh jit
# closure per call, which otherwise re-pays ~1s of XLA compile every call.
try:
    jax.config.update("jax_compilation_cache_dir", "/tmp/jax_cc_nnattn")
    jax.config.update("jax_persistent_cache_min_compile_time_secs", 0.0)
    jax.config.update("jax_persistent_cache_min_entry_size_bytes", 0)
except Exception:
    pass

F32 = mybir.dt.float32
F32R = mybir.dt.float32r
BF16 = mybir.dt.bfloat16
FP16 = mybir.dt.float16
I8 = mybir.dt.int8
NPBF16 = mybir.dt.np(BF16)
NPFP16 = mybir.dt.np(FP16)
AF = mybir.ActivationFunctionType
OP = mybir.AluOpType

T = 2048
D = 2048
HD = 128
NQ = 4          # query heads per core
TB = 512        # token block
NTB = T // TB   # 4
KT = D // 128   # 16 contraction tiles
ST = T // 128   # 16 s tiles
N_CORES = 8
WROWS = NQ * HD + HD + HD + NQ * HD   # 1280 packed weight rows
RMS_EPS = 1.1920928955078125e-07
INV_SQRT_HD = float(np.float32(1.0) / np.sqrt(np.float32(HD)))
NEG_BIG = -1.0e30


def _host_constants():
    t = np.arange(T, dtype=np.float32)
    inv_freq = (1.0 / 10000.0 ** (np.arange(0, HD, 2, dtype=np.float32) / HD))
    freqs = np.outer(t, inv_freq).astype(np.float32)        # [T, 64]
    cos_h = np.cos(freqs).T.astype(np.float32)              # [64, T]
    sin_h = np.sin(freqs).T.astype(np.float32)
    cosT = np.ascontiguousarray(np.concatenate([cos_h, cos_h], axis=0))
    sinT = np.ascontiguousarray(np.concatenate([sin_h, -sin_h], axis=0))
    s = np.arange(128)[:, None]
    u = np.arange(896)[None, :]
    maskadd = np.where(u >= s + 384, 0.0, NEG_BIG).astype(np.float32)
    ident = np.eye(128, dtype=np.float32)
    return cosT, sinT, maskadd, ident


def _build_nc():
    nc = bacc.Bacc("TRN2", target_bir_lowering=False, debug=False,
                   num_devices=N_CORES)

    # per-core external inputs: x bf16, weights int8 codes + fold scales
    xqd = nc.dram_tensor("xq", [T, 512], BF16, kind="ExternalInput")
    wpd = nc.dram_tensor("wpack", [WROWS // 2, D], I8, kind="ExternalInput")
    qgaind = nc.dram_tensor("qgain", [1, NQ], F32, kind="ExternalInput")
    scld = nc.dram_tensor("scl", [1, 2], F32, kind="ExternalInput")
    # int8 output codes + per-token f32 scales (halves fetch + zero-buffer
    # wire; the output is heavy-tailed per row, so scales go per token)
    outc = nc.dram_tensor("outc", [NQ * HD, T], I8, kind="ExternalOutput")
    oscld = nc.dram_tensor("oscl", [1, T], F32, kind="ExternalOutput")

    # NEFF-embedded rope tables (no wire bytes; fp16 keeps the BIR small,
    # rounding is ~2e-4).  Mask and identities are generated on device.
    cos_np, sin_np, _mask_np, _ident_np = _host_constants()
    cosd = nc.inline_tensor(cos_np.astype(NPFP16), name="cosT")
    sind = nc.inline_tensor(sin_np.astype(NPFP16), name="sinT")

    with nc.allow_low_precision(reason="bf16 wire + fp32r matmul pipeline"), \
         tile.TileContext(nc) as tc:
        with (
            tc.tile_pool(name="const", bufs=1) as constp,
            tc.tile_pool(name="acts", bufs=1) as actp,
            tc.tile_pool(name="weights", bufs=1) as wp,
            tc.tile_pool(name="psum_acc", bufs=6, space="PSUM") as psum_acc,
            tc.tile_pool(name="psum_small", bufs=2, space="PSUM") as psum_small,
            tc.tile_pool(name="dram", bufs=1, space="DRAM") as dramp,
        ):
            # ---- constants ----
            onesf = constp.tile([128, 1], F32)
            nc.vector.memset(onesf[:], 1.0)
            ones128 = constp.tile([128, 1], F32R)
            nc.scalar.copy(ones128[:], onesf[:])
            # causal mask: keep 0 where u >= s + 384, else NEG_BIG
            mask = constp.tile([128, 896], F32)
            nc.gpsimd.memset(mask[:], 0.0)
            nc.gpsimd.affine_select(out=mask[:], in_=mask[:],
                                    pattern=[[1, 896]], base=-384,
                                    channel_multiplier=-1,
                                    compare_op=OP.is_ge, fill=NEG_BIG)
            cosh = constp.tile([HD, T], FP16)
            nc.sync.dma_start(out=cosh[:], in_=cosd[:])
            cosb = constp.tile([HD, T], F32)
            nc.vector.tensor_copy(cosb[:], cosh[:])
            sinh = constp.tile([HD, T], FP16)
            nc.sync.dma_start(out=sinh[:], in_=sind[:])
            sinb = constp.tile([HD, T], F32)
            nc.vector.tensor_copy(sinb[:], sinh[:])
            # identities (transpose operands): diag(1) via affine_select
            onesb2 = constp.tile([128, 128], BF16)
            nc.vector.memset(onesb2[:], 1.0)
            identb = constp.tile([128, 128], BF16)
            nc.gpsimd.affine_select(out=identb[:], in_=onesb2[:],
                                    pattern=[[1, 128]], base=0,
                                    channel_multiplier=-1,
                                    compare_op=OP.is_equal, fill=0.0)
            onesf2 = constp.tile([128, 128], F32)
            nc.vector.memset(onesf2[:], 1.0)
            identf = constp.tile([128, 128], F32)
            nc.gpsimd.affine_select(out=identf[:], in_=onesf2[:],
                                    pattern=[[1, 128]], base=0,
                                    channel_multiplier=-1,
                                    compare_op=OP.is_equal, fill=0.0)
            qgain = constp.tile([1, NQ], F32)
            nc.sync.dma_start(out=qgain[:], in_=qgaind[:])
            scl = constp.tile([1, 2], F32)
            nc.sync.dma_start(out=scl[:], in_=scld[:])
            svb = constp.tile([128, 1], F32)
            nc.gpsimd.partition_broadcast(svb[:], scl[0:1, 0:1])
            spb = constp.tile([128, 1], F32)
            nc.gpsimd.partition_broadcast(spb[:], scl[0:1, 1:2])
            eps1 = constp.tile([1, 1], F32)
            nc.vector.memset(eps1[:], RMS_EPS)

            # ---- weight AllGather across the (b=0, b=1) pair ----
            wtb = dramp.tile([WROWS // 2, D], I8, name="wtb")
            wfull = dramp.tile([WROWS, D], I8, name="wfull")
            nc.sync.dma_start(out=wtb[:], in_=wpd[:])
            nc.gpsimd.collective_compute(
                "AllGather", OP.bypass,
                replica_groups=[[0, 4], [1, 5], [2, 6], [3, 7]],
                ins=[wtb[:].opt()], outs=[wfull[:].opt()])

            # ---- x transpose (on-device) + AllGather across head groups ----
            xtb = dramp.tile([512, T], BF16, name="xtb")
            xTd = dramp.tile([D, T], BF16, name="xTd")
            with tc.tile_pool(name="xtr", bufs=1) as xtrp:
                xTq = [xtrp.tile([128, T], BF16, name=f"xTq{fc}",
                                 tag=f"xTq{fc}") for fc in range(4)]
                for tr in range(ST):
                    xt_in = xtrp.tile([128, 512], BF16, name="xt_in",
                                      tag="xt_in", bufs=4)
                    nc.sync.dma_start(out=xt_in[:],
                                      in_=xqd[128 * tr:128 * (tr + 1), :])
                    for fc in range(4):
                        ps_t = psum_acc.tile([128, 128], BF16, name="ps_xt",
                                             tag="acc")
                        nc.tensor.transpose(
                            ps_t[:], xt_in[:, 128 * fc:128 * (fc + 1)],
                            identb[:])
                        nc.vector.tensor_copy(
                            xTq[fc][:, 128 * tr:128 * (tr + 1)], ps_t[:])
                for fc in range(4):
                    nc.sync.dma_start(out=xtb[128 * fc:128 * (fc + 1), :],
                                      in_=xTq[fc][:])
            nc.gpsimd.collective_compute(
                "AllGather", OP.bypass,
                replica_groups=[[0, 1, 2, 3], [4, 5, 6, 7]],
                ins=[xtb[:].opt()], outs=[xTd[:].opt()])

            # ---- weight transpose straight into effective weight tiles ----
            # wfull rows: q 0:512, k 512:640, v 640:768, p 768:1280
            wq_t = [wp.tile([128, NQ * HD], BF16, name=f"wq{ck}",
                            tag=f"wq{ck}") for ck in range(KT)]
            wk_t = [wp.tile([128, HD], BF16, name=f"wk{ck}", tag=f"wk{ck}")
                    for ck in range(KT)]
            wv_t = [wp.tile([128, HD], BF16, name=f"wv{ck}", tag=f"wv{ck}")
                    for ck in range(KT)]
            wp_t = [wp.tile([128, NQ * HD], BF16, name=f"wpj{ck}",
                            tag=f"wpj{ck}") for ck in range(KT)]
            with tc.tile_pool(name="wtr", bufs=1) as wtrp:
                for rt in range(10):
                    w_i8 = wtrp.tile([128, D], I8, name="w_i8",
                                     tag="w_i8", bufs=3)
                    nc.sync.dma_start(
                        out=w_i8[:], in_=wfull[128 * rt:128 * (rt + 1), :])
                    w_in = wtrp.tile([128, D], BF16, name="w_in",
                                     tag="w_in", bufs=3)
                    nc.vector.tensor_copy(w_in[:], w_i8[:])
                    if rt < 4:
                        dst, r = wq_t, rt
                    elif rt == 4:
                        dst, r = wk_t, 0
                    elif rt == 5:
                        dst, r = wv_t, 0
                    else:
                        dst, r = wp_t, rt - 6
                    for ck in range(KT):
                        ps_t = psum_acc.tile([128, 128], BF16, name="ps_wt",
                                             tag="acc")
                        nc.tensor.transpose(
                            ps_t[:], w_in[:, 128 * ck:128 * (ck + 1)],
                            identb[:])
                        nc.vector.tensor_copy(
                            dst[ck][:, 128 * r:128 * (r + 1)], ps_t[:])

            # ---- persistent activations ----
            qf = [actp.tile([128, T], F32R, name=f"qf{h}", tag=f"qf{h}")
                  for h in range(NQ)]
            kf = actp.tile([128, T], F32R, name="kf", tag="kf")
            vT = actp.tile([128, T], F32, name="vT", tag="vT")
            vs = [actp.tile([128, 128], F32R, name=f"vs{i}", tag=f"vs{i}")
                  for i in range(ST)]

            # ---- QKV projections + rmsnorm + rope ----
            with tc.tile_pool(name="qkv_tmp", bufs=2) as tp:
                for j in range(NTB):
                    js = slice(TB * j, TB * (j + 1))
                    # stream x k-tiles for this t-block from xTd (bf16)
                    xts = []
                    for ck in range(KT):
                        xt = tp.tile([128, TB], BF16, name="xt",
                                     tag="xt", bufs=4)
                        nc.sync.dma_start(
                            out=xt[:], in_=xTd[128 * ck:128 * (ck + 1), js])
                        xts.append(xt)
                    ps_o = [psum_acc.tile([128, TB], F32, name=f"ps_o{o}",
                                          tag="acc") for o in range(6)]
                    for ck in range(KT):
                        st, sp_ = (ck == 0), (ck == KT - 1)
                        for h in range(NQ):
                            nc.tensor.matmul(
                                ps_o[h][:],
                                wq_t[ck][:, 128 * h:128 * (h + 1)],
                                xts[ck][:], start=st, stop=sp_)
                        nc.tensor.matmul(ps_o[4][:], wk_t[ck][:], xts[ck][:],
                                         start=st, stop=sp_)
                        nc.tensor.matmul(ps_o[5][:], wv_t[ck][:], xts[ck][:],
                                         start=st, stop=sp_)

                    # v: evict straight to vT, folding the int8 scale s_v
                    nc.vector.tensor_scalar(out=vT[:, js], in0=ps_o[5][:],
                                            scalar1=svb[0:128, 0:1],
                                            scalar2=None, op0=OP.mult)

                    # q heads and k: rmsnorm + rope
                    for o in range(5):
                        is_q = o < NQ
                        raw = tp.tile([128, TB], F32, name="raw", tag="raw",
                                      bufs=3)
                        nc.scalar.copy(raw[:], ps_o[o][:])
                        sq = tp.tile([128, TB], F32R, name="sq", tag="sq",
                                     bufs=2)
                        nc.vector.tensor_tensor(out=sq[:], in0=raw[:],
                                                in1=raw[:], op=OP.mult)
                        ps_r = psum_small.tile([1, TB], F32, name="ps_r",
                                               tag="small")
                        nc.tensor.matmul(ps_r[:], ones128[:], sq[:],
                                         start=True, stop=True)
                        rsq = tp.tile([1, TB], F32, name="rsq", tag="rsq",
                                      bufs=2)
                        nc.scalar.activation(rsq[:], ps_r[:], AF.Sqrt,
                                             bias=eps1[0:1, 0:1],
                                             scale=1.0 / HD)
                        rinv = tp.tile([1, TB], F32, name="rinv", tag="rinv",
                                       bufs=2)
                        nc.vector.reciprocal(rinv[:], rsq[:])
                        rsc = tp.tile([1, TB], F32R, name="rsc", tag="rsc",
                                      bufs=2)
                        if is_q:
                            nc.vector.tensor_scalar(
                                out=rsc[:], in0=rinv[:],
                                scalar1=qgain[0:1, o:o + 1], scalar2=None,
                                op0=OP.mult)
                        else:
                            nc.scalar.copy(rsc[:], rinv[:])
                        rb_s = tp.tile([128, TB], F32, name="rb_s",
                                       tag="rb_s", bufs=2)
                        nc.gpsimd.partition_broadcast(rb_s[:],
                                                      rsc[:].bitcast(F32))
                        # rope: rawsw = halves of raw swapped; sinb has -sin
                        # in its high half, so ro = raw*cos + rawsw*sin.
                        rawsw = tp.tile([128, TB], F32, name="rawsw",
                                        tag="rawsw", bufs=2)
                        nc.scalar.copy(rawsw[0:64, :], raw[64:128, :])
                        nc.scalar.copy(rawsw[64:128, :], raw[0:64, :])
                        rock = tp.tile([128, TB], F32, name="rock",
                                       tag="rock", bufs=2)
                        nc.vector.tensor_tensor(out=rock[:], in0=raw[:],
                                                in1=cosb[:, js], op=OP.mult)
                        rask = tp.tile([128, TB], F32, name="rask",
                                       tag="rask", bufs=2)
                        nc.vector.tensor_tensor(out=rask[:], in0=rawsw[:],
                                                in1=sinb[:, js], op=OP.mult)
                        ro = tp.tile([128, TB], F32, name="ro", tag="ro",
                                     bufs=2)
                        nc.vector.tensor_tensor(out=ro[:], in0=rock[:],
                                                in1=rask[:], op=OP.add)
                        dst = qf[o][:, js] if is_q else kf[:, js]
                        nc.vector.tensor_tensor(out=dst, in0=ro[:],
                                                in1=rb_s[:], op=OP.mult)

            # v transposed tiles [s, dh] for the attn@v matmul
            with tc.tile_pool(name="vtr", bufs=2) as vtrp:
                for i in range(ST):
                    ps_t = psum_acc.tile([128, 128], F32, name="ps_vt",
                                         tag="acc")
                    nc.tensor.transpose(ps_t[:], vT[:, 128 * i:128 * (i + 1)],
                                        identf[:])
                    nc.scalar.copy(vs[i][:], ps_t[:])

            # ---- SDPA + _xsa per t-block, then one AllGather + proj ----
            ybounce = dramp.tile([NQ * HD, T], BF16, name="ybounce")
            yfull = dramp.tile([4 * NQ * HD, T], BF16, name="yfull")

            with tc.tile_pool(name="sdpa", bufs=2) as sp:
                for j in range(NTB):
                    js = slice(TB * j, TB * (j + 1))
                    n_i = 4 * j + 4
                    denr = sp.tile([1, TB], F32, name="denr", tag="denr",
                                   bufs=2)
                    for h in range(NQ):
                        ps_y = psum_acc.tile([128, TB], F32, name="ps_y",
                                             tag="acc")
                        ps_z = psum_small.tile([1, TB], F32, name="ps_z",
                                               tag="small")
                        for i in range(n_i):
                            ps_s = psum_acc.tile([128, TB], F32, name="ps_s",
                                                 tag="acc")
                            nc.tensor.matmul(
                                ps_s[:], kf[:, 128 * i:128 * (i + 1)],
                                qf[h][:, js], start=True, stop=True)
                            if i >= 4 * j:
                                off = 128 * (i - 4 * j)
                                u0 = 384 - off
                                nc.vector.tensor_tensor(
                                    out=ps_s[:], in0=ps_s[:],
                                    in1=mask[:, u0:u0 + TB], op=OP.add)
                            et = sp.tile([128, TB], F32R, name="et",
                                         tag=f"et{i & 1}", bufs=2)
                            nc.scalar.activation(et[:], ps_s[:], AF.Exp,
                                                 scale=INV_SQRT_HD)
                            st, spp = (i == 0), (i == n_i - 1)
                            nc.tensor.matmul(ps_z[:], ones128[:], et[:],
                                             start=st, stop=spp,
                                             skip_group_check=True)
                            nc.tensor.matmul(ps_y[:], vs[i][:], et[:],
                                             start=st, stop=spp,
                                             skip_group_check=True)
                        # epilogue for (h, j)
                        y_h = sp.tile([128, TB], F32, name="y_h", tag="y_h",
                                      bufs=2)
                        nc.scalar.copy(y_h[:], ps_y[:])
                        if h == 0:
                            vsq = sp.tile([128, TB], F32R, name="vsq",
                                          tag="vsq", bufs=1)
                            nc.vector.tensor_tensor(out=vsq[:], in0=vT[:, js],
                                                    in1=vT[:, js],
                                                    op=OP.mult)
                            ps_d = psum_small.tile([1, TB], F32, name="ps_d",
                                                   tag="small")
                            nc.tensor.matmul(ps_d[:], ones128[:], vsq[:],
                                             start=True, stop=True)
                            den = sp.tile([1, TB], F32, name="den", tag="den",
                                          bufs=2)
                            nc.vector.tensor_scalar(out=den[:], in0=ps_d[:],
                                                    scalar1=1e-24,
                                                    scalar2=None, op0=OP.max)
                            nc.vector.reciprocal(denr[:], den[:])
                        zinv = sp.tile([1, TB], F32, name="zinv", tag="zinv",
                                       bufs=2)
                        nc.vector.reciprocal(zinv[:], ps_z[:])
                        zr = sp.tile([1, TB], F32R, name="zr", tag="zr",
                                     bufs=2)
                        nc.scalar.copy(zr[:], zinv[:])
                        yv = sp.tile([128, TB], F32R, name="yv", tag="yv",
                                     bufs=1)
                        nc.vector.tensor_tensor(out=yv[:], in0=y_h[:],
                                                in1=vT[:, js], op=OP.mult)
                        ps_dot = psum_small.tile([1, TB], F32, name="ps_dot",
                                                 tag="small")
                        nc.tensor.matmul(ps_dot[:], ones128[:], yv[:],
                                         start=True, stop=True)
                        c1 = sp.tile([1, TB], F32, name="c1", tag="c1",
                                     bufs=2)
                        nc.vector.tensor_tensor(out=c1[:], in0=ps_dot[:],
                                                in1=denr[:], op=OP.mult)
                        c2 = sp.tile([1, TB], F32R, name="c2", tag="c2",
                                     bufs=2)
                        nc.vector.tensor_tensor(out=c2[:], in0=c1[:],
                                                in1=zinv[:], op=OP.mult)
                        zb_s = sp.tile([128, TB], F32, name="zb_s",
                                       tag="zb_s", bufs=1)
                        cb_s = sp.tile([128, TB], F32, name="cb_s",
                                       tag="cb_s", bufs=1)
                        nc.gpsimd.partition_broadcast(zb_s[:],
                                                      zr[:].bitcast(F32))
                        nc.gpsimd.partition_broadcast(cb_s[:],
                                                      c2[:].bitcast(F32))
                        t1 = sp.tile([128, TB], F32, name="t1", tag="t1",
                                     bufs=1)
                        t2 = sp.tile([128, TB], F32, name="t2", tag="t2",
                                     bufs=1)
                        nc.vector.tensor_tensor(out=t1[:], in0=y_h[:],
                                                in1=zb_s[:], op=OP.mult)
                        nc.vector.tensor_tensor(out=t2[:], in0=vT[:, js],
                                                in1=cb_s[:], op=OP.mult)
                        yfin = sp.tile([128, TB], BF16, name="yfin",
                                       tag="yfin", bufs=2)
                        nc.vector.tensor_tensor(out=yfin[:], in0=t1[:],
                                                in1=t2[:], op=OP.subtract)
                        nc.sync.dma_start(
                            out=ybounce[128 * h:128 * (h + 1), js],
                            in_=yfin[:])
            nc.gpsimd.collective_compute(
                "AllGather", OP.bypass,
                replica_groups=[[0, 1, 2, 3], [4, 5, 6, 7]],
                ins=[ybounce[:].opt()], outs=[yfull[:].opt()])

            # ---- output projection (row-sharded: 512 out cols/core) ----
            # Accumulate the full f32 result in SBUF, then int8-quantize with
            # per-row scales (round-to-nearest + saturation on the convert).
            with tc.tile_pool(name="proj", bufs=2) as pp:
                ofull = [pp.tile([128, T], F32, name=f"ofull{o}",
                                 tag=f"ofull{o}", bufs=1) for o in range(4)]
                for j in range(NTB):
                    js = slice(TB * j, TB * (j + 1))
                    ps_p = [psum_acc.tile([128, TB], F32, name=f"ps_p{o}",
                                          tag="acc") for o in range(4)]
                    for ck in range(KT):
                        yt = pp.tile([128, TB], BF16, name="yt", tag="yt",
                                     bufs=4)
                        nc.sync.dma_start(
                            out=yt[:],
                            in_=yfull[128 * ck:128 * (ck + 1), js])
                        st, spp = (ck == 0), (ck == KT - 1)
                        for o in range(4):
                            nc.tensor.matmul(
                                ps_p[o][:],
                                wp_t[ck][:, 128 * o:128 * (o + 1)],
                                yt[:], start=st, stop=spp)
                    for o in range(4):
                        nc.vector.tensor_scalar(out=ofull[o][:, js],
                                                in0=ps_p[o][:],
                                                scalar1=spb[0:128, 0:1],
                                                scalar2=None, op0=OP.mult)
                # per-token absmax over all 512 out rows (partition all-
                # reduce per o-tile, then max across the 4 tiles)
                am = pp.tile([128, T], F32, name="am", tag="am", bufs=1)
                am2 = pp.tile([128, T], F32, name="am2", tag="am2", bufs=1)
                nc.gpsimd.partition_all_reduce(
                    am[:], ofull[0][:], channels=128,
                    reduce_op=bass_isa.ReduceOp.absmax)
                for o in range(1, 4):
                    nc.gpsimd.partition_all_reduce(
                        am2[:], ofull[o][:], channels=128,
                        reduce_op=bass_isa.ReduceOp.absmax)
                    nc.vector.tensor_tensor(out=am[:], in0=am[:],
                                            in1=am2[:], op=OP.max)
                nc.vector.tensor_scalar(out=am[:], in0=am[:], scalar1=1e-30,
                                        scalar2=None, op0=OP.max)
                osc = pp.tile([1, T], F32, name="osc", tag="osc", bufs=1)
                nc.vector.tensor_scalar(out=osc[:], in0=am[0:1, :],
                                        scalar1=1.0 / 127.0, scalar2=None,
                                        op0=OP.mult)
                nc.sync.dma_start(out=oscld[:], in_=osc[:])
                rsc = pp.tile([128, T], F32, name="rsc2", tag="rsc2", bufs=1)
                nc.vector.reciprocal(rsc[:], am[:])
                nc.vector.tensor_scalar(out=rsc[:], in0=rsc[:], scalar1=127.0,
                                        scalar2=None, op0=OP.mult)
                for o in range(4):
                    codes = pp.tile([128, T], I8, name="codes", tag="codes",
                                    bufs=2)
                    nc.vector.tensor_tensor(out=codes[:], in0=ofull[o][:],
                                            in1=rsc[:], op=OP.mult)
                    nc.sync.dma_start(out=outc[128 * o:128 * (o + 1), :],
                                      in_=codes[:])

    nc.compile()
    return nc


_NC = None


def _get_nc():
    global _NC
    if _NC is None:
        _NC = _build_nc()
    return _NC


def _weight_codes(w, sf):
    """Host-side AnnealedBitLinear effective weight (f32, bitwise identical
    quantization decisions to the reference) + symmetric int8 encoding.
    Returns (codes int8, scale f32).  q/k scales never leave the host
    (rmsnorm makes q and k scale-invariant); s_v and s_p are folded back on
    device."""
    w = np.asarray(w, dtype=np.float32)
    wabs = np.abs(w)
    scale = np.clip(wabs.mean(axis=1, keepdims=True, dtype=np.float32),
                    1e-8, None).astype(np.float32)
    cond = wabs > (np.float32(0.7) * scale)
    # w_e = (1-sf)*w everywhere, += sf*copysign(scale, w) where cond
    # (identical quantization decisions; wabs buffer reused for w_e)
    w_e = np.multiply(w, np.float32(1.0 - sf), out=wabs)
    np.add(w_e, np.copysign(scale * sf, w), out=w_e, where=cond)
    # exact absmax without materializing |w_e|; scale maps max to +/-127
    # exactly, so no clip is needed before the int8 cast
    s = np.float32(max(max(w_e.max(), -w_e.min()) / 127.0, 1e-30))
    np.multiply(w_e, np.float32(1.0 / s), out=w_e)
    np.rint(w_e, out=w_e)
    return w_e.astype(np.int8), s


def _make_in_maps(x, step_fraction, w_q, w_k, w_v, w_proj, q_gain):
    x = np.asarray(x, dtype=np.float32)
    sf = np.float32(np.asarray(step_fraction, dtype=np.float32).reshape(-1)[0])
    q_gain = np.asarray(q_gain, dtype=np.float32)
    (wq_c, _), (wk_c, _), (wv_c, s_v), (wp_c, s_p) = \
        [_weight_codes(w, sf) for w in (w_q, w_k, w_v, w_proj)]
    xqs = [np.ascontiguousarray(
        x[c // 4][:, 512 * (c % 4):512 * (c % 4 + 1)]).astype(NPBF16)
        for c in range(N_CORES)]
    scl = np.array([[s_v, s_p]], dtype=np.float32)
    # packed per-head-group weight code blocks [1280, D] int8
    wpacks = []
    for h in range(4):
        wpacks.append(np.concatenate([
            wq_c[512 * h:512 * (h + 1), :],
            wk_c[128 * h:128 * (h + 1), :],
            wv_c[128 * h:128 * (h + 1), :],
            wp_c[512 * h:512 * (h + 1), :]], axis=0))
    in_maps = []
    half = WROWS // 2
    for c in range(N_CORES):
        b, h = divmod(c, 4)
        in_maps.append({
            "xq": xqs[c],
            "wpack": np.ascontiguousarray(
                wpacks[h][half * b:half * (b + 1), :]),
            "qgain": np.ascontiguousarray(q_gain[4 * h:4 * (h + 1)]
                                          .reshape(1, NQ)),
            "scl": scl,
        })
    return in_maps


def _assemble(results):
    out = np.empty((2, T, D), dtype=np.float32)

    for c in range(N_CORES):
        b, h = divmod(c, 4)
        codes = results[c]["outc"]            # [512, T] int8
        scl = results[c]["oscl"]              # [1, T] f32 per-token scale
        out[b][:, 512 * h:512 * (h + 1)] = \
            np.multiply(codes, scl, dtype=np.float32).T
    return out


def kernel(**inputs) -> np.ndarray:
    nc = _get_nc()
    in_maps = _make_in_maps(**inputs)
    res = bass_utils.run_bass_kernel_spmd(nc, in_maps,
                                          core_ids=list(range(N_CORES)))
    return _assemble(res.results)


def bench(**inputs):
    """Returns (output, BassKernelResults); tracing if the env supports it."""
    nc = _get_nc()
    in_maps = _make_in_maps(**inputs)
    try:
        res = bass_utils.run_bass_kernel_spmd(nc, in_maps,
                                              core_ids=list(range(N_CORES)),
                                              trace=True)
    except ModuleNotFoundError:
        res = bass_utils.run_bass_kernel_spmd(nc, in_maps,
                                              core_ids=list(range(N_CORES)))
    return _assemble(res.results), res


# revision 17
# speedup vs baseline: 1.1530x; 1.0423x over previous
"""Trainium2 Bass kernel for nn_CausalSelfAttention_60284160967096 (v4).

Sharding: 8 cores = 2 (batch) x 4 (kv-head groups).  Each core computes its
batch's attention for one kv-head (4 query heads), the Gram-Schmidt (_xsa)
correction, then an AllGather of y within the 4-core group and a row-sharded
output projection producing a 512-column slice of the output.

The axon tunnel (~44 MB/s) dominates wall time, so wire bytes are minimized:
  - x / weights / output cross the wire in bf16
  - x is shipped as per-core feature quarters in natural [T, 512] layout
    (16 MB total instead of 4x-duplicated 128 MB) and AllGathered on-device
    after an on-device PE transpose
  - weight slices are packed into one [1280, D] block per head group; each
    core ships HALF of it and an AllGather over (b=0,b=1) pairs rebuilds the
    full block (21 MB instead of 80 MB)
  - rope tables / causal mask / identity are inline_tensor NEFF constants
    (zero wire bytes)
The ternary-quantized *effective* weights are computed on the host in f32
(bitwise-identical quantization decisions to the reference; device-side
quantization of bf16-rounded weights flips ~0.14% of ternary decisions and
costs ~2% rel error).  SDPA/rmsnorm/rope/_xsa stay fp32/fp32r on device;
only wire-adjacent tensors are bf16.
"""

import numpy as np

import jax

import concourse.bass as bass
import concourse.bass_isa as bass_isa
import concourse.mybir as mybir
import concourse.tile as tile
from concourse import bacc, bass_utils

# Persistent XLA compilation cache: run_bass_kernel_spmd builds a fresh jit
# closure per call, which otherwise re-pays ~1s of XLA compile every call.
try:
    jax.config.update("jax_compilation_cache_dir", "/tmp/jax_cc_nnattn")
    jax.config.update("jax_persistent_cache_min_compile_time_secs", 0.0)
    jax.config.update("jax_persistent_cache_min_entry_size_bytes", 0)
except Exception:
    pass

F32 = mybir.dt.float32
F32R = mybir.dt.float32r
BF16 = mybir.dt.bfloat16
FP16 = mybir.dt.float16
I8 = mybir.dt.int8
NPBF16 = mybir.dt.np(BF16)
NPFP16 = mybir.dt.np(FP16)
AF = mybir.ActivationFunctionType
OP = mybir.AluOpType

T = 2048
D = 2048
HD = 128
NQ = 4          # query heads per core
TB = 512        # token block
NTB = T // TB   # 4
KT = D // 128   # 16 contraction tiles
ST = T // 128   # 16 s tiles
N_CORES = 8
WROWS = NQ * HD + HD + HD + NQ * HD   # 1280 packed weight rows
RMS_EPS = 1.1920928955078125e-07
INV_SQRT_HD = float(np.float32(1.0) / np.sqrt(np.float32(HD)))
NEG_BIG = -1.0e30


def _host_constants():
    t = np.arange(T, dtype=np.float32)
    inv_freq = (1.0 / 10000.0 ** (np.arange(0, HD, 2, dtype=np.float32) / HD))
    freqs = np.outer(t, inv_freq).astype(np.float32)        # [T, 64]
    cos_h = np.cos(freqs).T.astype(np.float32)              # [64, T]
    sin_h = np.sin(freqs).T.astype(np.float32)
    cosT = np.ascontiguousarray(np.concatenate([cos_h, cos_h], axis=0))
    sinT = np.ascontiguousarray(np.concatenate([sin_h, -sin_h], axis=0))
    s = np.arange(128)[:, None]
    u = np.arange(896)[None, :]
    maskadd = np.where(u >= s + 384, 0.0, NEG_BIG).astype(np.float32)
    ident = np.eye(128, dtype=np.float32)
    return cosT, sinT, maskadd, ident


def _build_nc():
    nc = bacc.Bacc("TRN2", target_bir_lowering=False, debug=False,
                   num_devices=N_CORES)

    # per-core external inputs: x bf16, weights int8 codes + fold scales
    xqd = nc.dram_tensor("xq", [T, 512], BF16, kind="ExternalInput")
    wpd = nc.dram_tensor("wpack", [WROWS // 2, D], I8, kind="ExternalInput")
    qgaind = nc.dram_tensor("qgain", [1, NQ], F32, kind="ExternalInput")
    scld = nc.dram_tensor("scl", [1, 2], F32, kind="ExternalInput")
    # int8 output codes + per-token f32 scales (halves fetch + zero-buffer
    # wire; the output is heavy-tailed per row, so scales go per token)
    outc = nc.dram_tensor("outc", [NQ * HD, T], I8, kind="ExternalOutput")
    oscld = nc.dram_tensor("oscl", [1, T], F32, kind="ExternalOutput")

    # NEFF-embedded rope tables (no wire bytes; fp16 keeps the BIR small,
    # rounding is ~2e-4).  Mask and identities are generated on device.
    cos_np, sin_np, _mask_np, _ident_np = _host_constants()
    cosd = nc.inline_tensor(cos_np.astype(NPFP16), name="cosT")
    sind = nc.inline_tensor(sin_np.astype(NPFP16), name="sinT")

    with nc.allow_low_precision(reason="bf16 wire + fp32r matmul pipeline"), \
         tile.TileContext(nc) as tc:
        with (
            tc.tile_pool(name="const", bufs=1) as constp,
            tc.tile_pool(name="acts", bufs=1) as actp,
            tc.tile_pool(name="weights", bufs=1) as wp,
            tc.tile_pool(name="psum_acc", bufs=6, space="PSUM") as psum_acc,
            tc.tile_pool(name="psum_small", bufs=2, space="PSUM") as psum_small,
            tc.tile_pool(name="dram", bufs=1, space="DRAM") as dramp,
        ):
            # ---- constants ----
            onesf = constp.tile([128, 1], F32)
            nc.vector.memset(onesf[:], 1.0)
            ones128 = constp.tile([128, 1], F32R)
            nc.scalar.copy(ones128[:], onesf[:])
            # causal mask: keep 0 where u >= s + 384, else NEG_BIG
            mask = constp.tile([128, 896], F32)
            nc.gpsimd.memset(mask[:], 0.0)
            nc.gpsimd.affine_select(out=mask[:], in_=mask[:],
                                    pattern=[[1, 896]], base=-384,
                                    channel_multiplier=-1,
                                    compare_op=OP.is_ge, fill=NEG_BIG)
            cosh = constp.tile([HD, T], FP16)
            nc.sync.dma_start(out=cosh[:], in_=cosd[:])
            cosb = constp.tile([HD, T], F32)
            nc.vector.tensor_copy(cosb[:], cosh[:])
            sinh = constp.tile([HD, T], FP16)
            nc.sync.dma_start(out=sinh[:], in_=sind[:])
            sinb = constp.tile([HD, T], F32)
            nc.vector.tensor_copy(sinb[:], sinh[:])
            # identities (transpose operands): diag(1) via affine_select
            onesb2 = constp.tile([128, 128], BF16)
            nc.vector.memset(onesb2[:], 1.0)
            identb = constp.tile([128, 128], BF16)
            nc.gpsimd.affine_select(out=identb[:], in_=onesb2[:],
                                    pattern=[[1, 128]], base=0,
                                    channel_multiplier=-1,
                                    compare_op=OP.is_equal, fill=0.0)
            onesf2 = constp.tile([128, 128], F32)
            nc.vector.memset(onesf2[:], 1.0)
            identf = constp.tile([128, 128], F32)
            nc.gpsimd.affine_select(out=identf[:], in_=onesf2[:],
                                    pattern=[[1, 128]], base=0,
                                    channel_multiplier=-1,
                                    compare_op=OP.is_equal, fill=0.0)
            qgain = constp.tile([1, NQ], F32)
            nc.sync.dma_start(out=qgain[:], in_=qgaind[:])
            scl = constp.tile([1, 2], F32)
            nc.sync.dma_start(out=scl[:], in_=scld[:])
            svb = constp.tile([128, 1], F32)
            nc.gpsimd.partition_broadcast(svb[:], scl[0:1, 0:1])
            spb = constp.tile([128, 1], F32)
            nc.gpsimd.partition_broadcast(spb[:], scl[0:1, 1:2])
            eps1 = constp.tile([1, 1], F32)
            nc.vector.memset(eps1[:], RMS_EPS)

            # ---- weight AllGather across the (b=0, b=1) pair ----
            wtb = dramp.tile([WROWS // 2, D], I8, name="wtb")
            wfull = dramp.tile([WROWS, D], I8, name="wfull")
            nc.sync.dma_start(out=wtb[:], in_=wpd[:])
            nc.gpsimd.collective_compute(
                "AllGather", OP.bypass,
                replica_groups=[[0, 4], [1, 5], [2, 6], [3, 7]],
                ins=[wtb[:].opt()], outs=[wfull[:].opt()])

            # ---- x transpose (on-device) + AllGather across head groups ----
            xtb = dramp.tile([512, T], BF16, name="xtb")
            xTd = dramp.tile([D, T], BF16, name="xTd")
            with tc.tile_pool(name="xtr", bufs=1) as xtrp:
                xTq = [xtrp.tile([128, T], BF16, name=f"xTq{fc}",
                                 tag=f"xTq{fc}") for fc in range(4)]
                for tr in range(ST):
                    xt_in = xtrp.tile([128, 512], BF16, name="xt_in",
                                      tag="xt_in", bufs=4)
                    nc.sync.dma_start(out=xt_in[:],
                                      in_=xqd[128 * tr:128 * (tr + 1), :])
                    for fc in range(4):
                        ps_t = psum_acc.tile([128, 128], BF16, name="ps_xt",
                                             tag="acc")
                        nc.tensor.transpose(
                            ps_t[:], xt_in[:, 128 * fc:128 * (fc + 1)],
                            identb[:])
                        nc.vector.tensor_copy(
                            xTq[fc][:, 128 * tr:128 * (tr + 1)], ps_t[:])
                for fc in range(4):
                    nc.sync.dma_start(out=xtb[128 * fc:128 * (fc + 1), :],
                                      in_=xTq[fc][:])
            nc.gpsimd.collective_compute(
                "AllGather", OP.bypass,
                replica_groups=[[0, 1, 2, 3], [4, 5, 6, 7]],
                ins=[xtb[:].opt()], outs=[xTd[:].opt()])

            # ---- weight transpose straight into effective weight tiles ----
            # wfull rows: q 0:512, k 512:640, v 640:768, p 768:1280
            wq_t = [wp.tile([128, NQ * HD], BF16, name=f"wq{ck}",
                            tag=f"wq{ck}") for ck in range(KT)]
            wk_t = [wp.tile([128, HD], BF16, name=f"wk{ck}", tag=f"wk{ck}")
                    for ck in range(KT)]
            wv_t = [wp.tile([128, HD], BF16, name=f"wv{ck}", tag=f"wv{ck}")
                    for ck in range(KT)]
            wp_t = [wp.tile([128, NQ * HD], BF16, name=f"wpj{ck}",
                            tag=f"wpj{ck}") for ck in range(KT)]
            with tc.tile_pool(name="wtr", bufs=1) as wtrp:
                for rt in range(10):
                    w_i8 = wtrp.tile([128, D], I8, name="w_i8",
                                     tag="w_i8", bufs=3)
                    nc.sync.dma_start(
                        out=w_i8[:], in_=wfull[128 * rt:128 * (rt + 1), :])
                    w_in = wtrp.tile([128, D], BF16, name="w_in",
                                     tag="w_in", bufs=3)
                    nc.vector.tensor_copy(w_in[:], w_i8[:])
                    if rt < 4:
                        dst, r = wq_t, rt
                    elif rt == 4:
                        dst, r = wk_t, 0
                    elif rt == 5:
                        dst, r = wv_t, 0
                    else:
                        dst, r = wp_t, rt - 6
                    for ck in range(KT):
                        ps_t = psum_acc.tile([128, 128], BF16, name="ps_wt",
                                             tag="acc")
                        nc.tensor.transpose(
                            ps_t[:], w_in[:, 128 * ck:128 * (ck + 1)],
                            identb[:])
                        nc.vector.tensor_copy(
                            dst[ck][:, 128 * r:128 * (r + 1)], ps_t[:])

            # ---- persistent activations ----
            qf = [actp.tile([128, T], F32R, name=f"qf{h}", tag=f"qf{h}")
                  for h in range(NQ)]
            kf = actp.tile([128, T], F32R, name="kf", tag="kf")
            vT = actp.tile([128, T], F32, name="vT", tag="vT")
            vs = [actp.tile([128, 128], F32R, name=f"vs{i}", tag=f"vs{i}")
                  for i in range(ST)]

            # ---- QKV projections + rmsnorm + rope ----
            with tc.tile_pool(name="qkv_tmp", bufs=2) as tp:
                for j in range(NTB):
                    js = slice(TB * j, TB * (j + 1))
                    # stream x k-tiles for this t-block from xTd (bf16)
                    xts = []
                    for ck in range(KT):
                        xt = tp.tile([128, TB], BF16, name="xt",
                                     tag="xt", bufs=4)
                        nc.sync.dma_start(
                            out=xt[:], in_=xTd[128 * ck:128 * (ck + 1), js])
                        xts.append(xt)
                    ps_o = [psum_acc.tile([128, TB], F32, name=f"ps_o{o}",
                                          tag="acc") for o in range(6)]
                    for ck in range(KT):
                        st, sp_ = (ck == 0), (ck == KT - 1)
                        for h in range(NQ):
                            nc.tensor.matmul(
                                ps_o[h][:],
                                wq_t[ck][:, 128 * h:128 * (h + 1)],
                                xts[ck][:], start=st, stop=sp_)
                        nc.tensor.matmul(ps_o[4][:], wk_t[ck][:], xts[ck][:],
                                         start=st, stop=sp_)
                        nc.tensor.matmul(ps_o[5][:], wv_t[ck][:], xts[ck][:],
                                         start=st, stop=sp_)

                    # v: evict straight to vT, folding the int8 scale s_v
                    nc.vector.tensor_scalar(out=vT[:, js], in0=ps_o[5][:],
                                            scalar1=svb[0:128, 0:1],
                                            scalar2=None, op0=OP.mult)

                    # q heads and k: rmsnorm + rope
                    for o in range(5):
                        is_q = o < NQ
                        raw = tp.tile([128, TB], F32, name="raw", tag="raw",
                                      bufs=3)
                        nc.scalar.copy(raw[:], ps_o[o][:])
                        sq = tp.tile([128, TB], F32R, name="sq", tag="sq",
                                     bufs=2)
                        nc.vector.tensor_tensor(out=sq[:], in0=raw[:],
                                                in1=raw[:], op=OP.mult)
                        ps_r = psum_small.tile([1, TB], F32, name="ps_r",
                                               tag="small")
                        nc.tensor.matmul(ps_r[:], ones128[:], sq[:],
                                         start=True, stop=True)
                        rsq = tp.tile([1, TB], F32, name="rsq", tag="rsq",
                                      bufs=2)
                        nc.scalar.activation(rsq[:], ps_r[:], AF.Sqrt,
                                             bias=eps1[0:1, 0:1],
                                             scale=1.0 / HD)
                        rinv = tp.tile([1, TB], F32, name="rinv", tag="rinv",
                                       bufs=2)
                        nc.vector.reciprocal(rinv[:], rsq[:])
                        rsc = tp.tile([1, TB], F32R, name="rsc", tag="rsc",
                                      bufs=2)
                        if is_q:
                            nc.vector.tensor_scalar(
                                out=rsc[:], in0=rinv[:],
                                scalar1=qgain[0:1, o:o + 1], scalar2=None,
                                op0=OP.mult)
                        else:
                            nc.scalar.copy(rsc[:], rinv[:])
                        rb_s = tp.tile([128, TB], F32, name="rb_s",
                                       tag="rb_s", bufs=2)
                        nc.gpsimd.partition_broadcast(rb_s[:],
                                                      rsc[:].bitcast(F32))
                        # rope: rawsw = halves of raw swapped; sinb has -sin
                        # in its high half, so ro = raw*cos + rawsw*sin.
                        rawsw = tp.tile([128, TB], F32, name="rawsw",
                                        tag="rawsw", bufs=2)
                        nc.scalar.copy(rawsw[0:64, :], raw[64:128, :])
                        nc.scalar.copy(rawsw[64:128, :], raw[0:64, :])
                        rock = tp.tile([128, TB], F32, name="rock",
                                       tag="rock", bufs=2)
                        nc.vector.tensor_tensor(out=rock[:], in0=raw[:],
                                                in1=cosb[:, js], op=OP.mult)
                        rask = tp.tile([128, TB], F32, name="rask",
                                       tag="rask", bufs=2)
                        nc.vector.tensor_tensor(out=rask[:], in0=rawsw[:],
                                                in1=sinb[:, js], op=OP.mult)
                        ro = tp.tile([128, TB], F32, name="ro", tag="ro",
                                     bufs=2)
                        nc.vector.tensor_tensor(out=ro[:], in0=rock[:],
                                                in1=rask[:], op=OP.add)
                        dst = qf[o][:, js] if is_q else kf[:, js]
                        nc.vector.tensor_tensor(out=dst, in0=ro[:],
                                                in1=rb_s[:], op=OP.mult)

            # v transposed tiles [s, dh] for the attn@v matmul
            with tc.tile_pool(name="vtr", bufs=2) as vtrp:
                for i in range(ST):
                    ps_t = psum_acc.tile([128, 128], F32, name="ps_vt",
                                         tag="acc")
                    nc.tensor.transpose(ps_t[:], vT[:, 128 * i:128 * (i + 1)],
                                        identf[:])
                    nc.scalar.copy(vs[i][:], ps_t[:])

            # ---- SDPA + _xsa per t-block, then one AllGather + proj ----
            ybounce = dramp.tile([NQ * HD, T], BF16, name="ybounce")
            yfull = dramp.tile([4 * NQ * HD, T], BF16, name="yfull")

            with tc.tile_pool(name="sdpa", bufs=2) as sp:
                for j in range(NTB):
                    js = slice(TB * j, TB * (j + 1))
                    n_i = 4 * j + 4
                    denr = sp.tile([1, TB], F32, name="denr", tag="denr",
                                   bufs=2)
                    for h in range(NQ):
                        ps_y = psum_acc.tile([128, TB], F32, name="ps_y",
                                             tag="acc")
                        ps_z = psum_small.tile([1, TB], F32, name="ps_z",
                                               tag="small")
                        for i in range(n_i):
                            ps_s = psum_acc.tile([128, TB], F32, name="ps_s",
                                                 tag="acc")
                            nc.tensor.matmul(
                                ps_s[:], kf[:, 128 * i:128 * (i + 1)],
                                qf[h][:, js], start=True, stop=True)
                            if i >= 4 * j:
                                off = 128 * (i - 4 * j)
                                u0 = 384 - off
                                nc.vector.tensor_tensor(
                                    out=ps_s[:], in0=ps_s[:],
                                    in1=mask[:, u0:u0 + TB], op=OP.add)
                            et = sp.tile([128, TB], F32R, name="et",
                                         tag=f"et{i & 1}", bufs=2)
                            nc.scalar.activation(et[:], ps_s[:], AF.Exp,
                                                 scale=INV_SQRT_HD)
                            st, spp = (i == 0), (i == n_i - 1)
                            nc.tensor.matmul(ps_z[:], ones128[:], et[:],
                                             start=st, stop=spp,
                                             skip_group_check=True)
                            nc.tensor.matmul(ps_y[:], vs[i][:], et[:],
                                             start=st, stop=spp,
                                             skip_group_check=True)
                        # epilogue for (h, j)
                        y_h = sp.tile([128, TB], F32, name="y_h", tag="y_h",
                                      bufs=2)
                        nc.scalar.copy(y_h[:], ps_y[:])
                        if h == 0:
                            vsq = sp.tile([128, TB], F32R, name="vsq",
                                          tag="vsq", bufs=1)
                            nc.vector.tensor_tensor(out=vsq[:], in0=vT[:, js],
                                                    in1=vT[:, js],
                                                    op=OP.mult)
                            ps_d = psum_small.tile([1, TB], F32, name="ps_d",
                                                   tag="small")
                            nc.tensor.matmul(ps_d[:], ones128[:], vsq[:],
                                             start=True, stop=True)
                            den = sp.tile([1, TB], F32, name="den", tag="den",
                                          bufs=2)
                            nc.vector.tensor_scalar(out=den[:], in0=ps_d[:],
                                                    scalar1=1e-24,
                                                    scalar2=None, op0=OP.max)
                            nc.vector.reciprocal(denr[:], den[:])
                        zinv = sp.tile([1, TB], F32, name="zinv", tag="zinv",
                                       bufs=2)
                        nc.vector.reciprocal(zinv[:], ps_z[:])
                        zr = sp.tile([1, TB], F32R, name="zr", tag="zr",
                                     bufs=2)
                        nc.scalar.copy(zr[:], zinv[:])
                        yv = sp.tile([128, TB], F32R, name="yv", tag="yv",
                                     bufs=1)
                        nc.vector.tensor_tensor(out=yv[:], in0=y_h[:],
                                                in1=vT[:, js], op=OP.mult)
                        ps_dot = psum_small.tile([1, TB], F32, name="ps_dot",
                                                 tag="small")
                        nc.tensor.matmul(ps_dot[:], ones128[:], yv[:],
                                         start=True, stop=True)
                        c1 = sp.tile([1, TB], F32, name="c1", tag="c1",
                                     bufs=2)
                        nc.vector.tensor_tensor(out=c1[:], in0=ps_dot[:],
                                                in1=denr[:], op=OP.mult)
                        c2 = sp.tile([1, TB], F32R, name="c2", tag="c2",
                                     bufs=2)
                        nc.vector.tensor_tensor(out=c2[:], in0=c1[:],
                                                in1=zinv[:], op=OP.mult)
                        zb_s = sp.tile([128, TB], F32, name="zb_s",
                                       tag="zb_s", bufs=1)
                        cb_s = sp.tile([128, TB], F32, name="cb_s",
                                       tag="cb_s", bufs=1)
                        nc.gpsimd.partition_broadcast(zb_s[:],
                                                      zr[:].bitcast(F32))
                        nc.gpsimd.partition_broadcast(cb_s[:],
                                                      c2[:].bitcast(F32))
                        t1 = sp.tile([128, TB], F32, name="t1", tag="t1",
                                     bufs=1)
                        t2 = sp.tile([128, TB], F32, name="t2", tag="t2",
                                     bufs=1)
                        nc.vector.tensor_tensor(out=t1[:], in0=y_h[:],
                                                in1=zb_s[:], op=OP.mult)
                        nc.vector.tensor_tensor(out=t2[:], in0=vT[:, js],
                                                in1=cb_s[:], op=OP.mult)
                        yfin = sp.tile([128, TB], BF16, name="yfin",
                                       tag="yfin", bufs=2)
                        nc.vector.tensor_tensor(out=yfin[:], in0=t1[:],
                                                in1=t2[:], op=OP.subtract)
                        nc.sync.dma_start(
                            out=ybounce[128 * h:128 * (h + 1), js],
                            in_=yfin[:])
            nc.gpsimd.collective_compute(
                "AllGather", OP.bypass,
                replica_groups=[[0, 1, 2, 3], [4, 5, 6, 7]],
                ins=[ybounce[:].opt()], outs=[yfull[:].opt()])

            # ---- output projection (row-sharded: 512 out cols/core) ----
            # Accumulate the full f32 result in SBUF, then int8-quantize with
            # per-row scales (round-to-nearest + saturation on the convert).
            with tc.tile_pool(name="proj", bufs=2) as pp:
                ofull = [pp.tile([128, T], F32, name=f"ofull{o}",
                                 tag=f"ofull{o}", bufs=1) for o in range(4)]
                for j in range(NTB):
                    js = slice(TB * j, TB * (j + 1))
                    ps_p = [psum_acc.tile([128, TB], F32, name=f"ps_p{o}",
                                          tag="acc") for o in range(4)]
                    for ck in range(KT):
                        yt = pp.tile([128, TB], BF16, name="yt", tag="yt",
                                     bufs=4)
                        nc.sync.dma_start(
                            out=yt[:],
                            in_=yfull[128 * ck:128 * (ck + 1), js])
                        st, spp = (ck == 0), (ck == KT - 1)
                        for o in range(4):
                            nc.tensor.matmul(
                                ps_p[o][:],
                                wp_t[ck][:, 128 * o:128 * (o + 1)],
                                yt[:], start=st, stop=spp)
                    for o in range(4):
                        nc.vector.tensor_scalar(out=ofull[o][:, js],
                                                in0=ps_p[o][:],
                                                scalar1=spb[0:128, 0:1],
                                                scalar2=None, op0=OP.mult)
                # per-token absmax over all 512 out rows (partition all-
                # reduce per o-tile, then max across the 4 tiles)
                am = pp.tile([128, T], F32, name="am", tag="am", bufs=1)
                am2 = pp.tile([128, T], F32, name="am2", tag="am2", bufs=1)
                nc.gpsimd.partition_all_reduce(
                    am[:], ofull[0][:], channels=128,
                    reduce_op=bass_isa.ReduceOp.absmax)
                for o in range(1, 4):
                    nc.gpsimd.partition_all_reduce(
                        am2[:], ofull[o][:], channels=128,
                        reduce_op=bass_isa.ReduceOp.absmax)
                    nc.vector.tensor_tensor(out=am[:], in0=am[:],
                                            in1=am2[:], op=OP.max)
                nc.vector.tensor_scalar(out=am[:], in0=am[:], scalar1=1e-30,
                                        scalar2=None, op0=OP.max)
                osc = pp.tile([1, T], F32, name="osc", tag="osc", bufs=1)
                nc.vector.tensor_scalar(out=osc[:], in0=am[0:1, :],
                                        scalar1=1.0 / 127.0, scalar2=None,
                                        op0=OP.mult)
                nc.sync.dma_start(out=oscld[:], in_=osc[:])
                rsc = pp.tile([128, T], F32, name="rsc2", tag="rsc2", bufs=1)
                nc.vector.reciprocal(rsc[:], am[:])
                nc.vector.tensor_scalar(out=rsc[:], in0=rsc[:], scalar1=127.0,
                                        scalar2=None, op0=OP.mult)
                for o in range(4):
                    codes = pp.tile([128, T], I8, name="codes", tag="codes",
                                    bufs=2)
                    nc.vector.tensor_tensor(out=codes[:], in0=ofull[o][:],
                                            in1=rsc[:], op=OP.mult)
                    nc.sync.dma_start(out=outc[128 * o:128 * (o + 1), :],
                                      in_=codes[:])

    nc.compile()
    return nc


_NC = None


def _get_nc():
    global _NC
    if _NC is None:
        _NC = _build_nc()
    return _NC


def _weight_codes(w, sf):
    """Host-side AnnealedBitLinear effective weight (f32, bitwise identical
    quantization decisions to the reference) + symmetric int8 encoding.
    Returns (codes int8, scale f32).  q/k scales never leave the host
    (rmsnorm makes q and k scale-invariant); s_v and s_p are folded back on
    device."""
    w = np.asarray(w, dtype=np.float32)
    wabs = np.abs(w)
    scale = np.clip(wabs.mean(axis=1, keepdims=True, dtype=np.float32),
                    1e-8, None).astype(np.float32)
    cond = wabs > (np.float32(0.7) * scale)
    # w_e = (1-sf)*w everywhere, += sf*copysign(scale, w) where cond
    # (identical quantization decisions; wabs buffer reused for w_e)
    w_e = np.multiply(w, np.float32(1.0 - sf), out=wabs)
    np.add(w_e, np.copysign(scale * sf, w), out=w_e, where=cond)
    # exact absmax without materializing |w_e|; scale maps max to +/-127
    # exactly, so no clip is needed before the int8 cast
    s = np.float32(max(max(w_e.max(), -w_e.min()) / 127.0, 1e-30))
    np.multiply(w_e, np.float32(1.0 / s), out=w_e)
    np.rint(w_e, out=w_e)
    return w_e.astype(np.int8), s


def _make_in_maps(x, step_fraction, w_q, w_k, w_v, w_proj, q_gain):
    x = np.asarray(x, dtype=np.float32)
    sf = np.float32(np.asarray(step_fraction, dtype=np.float32).reshape(-1)[0])
    q_gain = np.asarray(q_gain, dtype=np.float32)
    (wq_c, _), (wk_c, _), (wv_c, s_v), (wp_c, s_p) = \
        [_weight_codes(w, sf) for w in (w_q, w_k, w_v, w_proj)]
    xqs = [np.ascontiguousarray(
        x[c // 4][:, 512 * (c % 4):512 * (c % 4 + 1)]).astype(NPBF16)
        for c in range(N_CORES)]
    scl = np.array([[s_v, s_p]], dtype=np.float32)
    # The packed block per head group is [wq(512); wk(128); wv(128); wp(512)]
    # and the b=0/b=1 halves split at row 640 — exactly the wq+wk | wv+wp
    # boundary, so each core's half is built directly from two slices.
    in_maps = []
    for c in range(N_CORES):
        b, h = divmod(c, 4)
        if b == 0:
            wpk = np.concatenate([wq_c[512 * h:512 * (h + 1), :],
                                  wk_c[128 * h:128 * (h + 1), :]], axis=0)
        else:
            wpk = np.concatenate([wv_c[128 * h:128 * (h + 1), :],
                                  wp_c[512 * h:512 * (h + 1), :]], axis=0)
        in_maps.append({
            "xq": xqs[c],
            "wpack": wpk,
            "qgain": np.ascontiguousarray(q_gain[4 * h:4 * (h + 1)]
                                          .reshape(1, NQ)),
            "scl": scl,
        })
    return in_maps


def _assemble(results):
    out = np.empty((2, T, D), dtype=np.float32)

    for c in range(N_CORES):
        b, h = divmod(c, 4)
        codes = results[c]["outc"]            # [512, T] int8
        scl = results[c]["oscl"]              # [1, T] f32 per-token scale
        out[b][:, 512 * h:512 * (h + 1)] = \
            np.multiply(codes, scl, dtype=np.float32).T
    return out


def kernel(**inputs) -> np.ndarray:
    nc = _get_nc()
    in_maps = _make_in_maps(**inputs)
    res = bass_utils.run_bass_kernel_spmd(nc, in_maps,
                                          core_ids=list(range(N_CORES)))
    return _assemble(res.results)


def bench(**inputs):
    """Returns (output, BassKernelResults); tracing if the env supports it."""
    nc = _get_nc()
    in_maps = _make_in_maps(**inputs)
    try:
        res = bass_utils.run_bass_kernel_spmd(nc, in_maps,
                                              core_ids=list(range(N_CORES)),
                                              trace=True)
    except ModuleNotFoundError:
        res = bass_utils.run_bass_kernel_spmd(nc, in_maps,
                                              core_ids=list(range(N_CORES)))
    return _assemble(res.results), res
